# revision 1
# baseline (speedup 1.0000x reference)
"""Trainium2 Bass kernel v3 for AttentionWithBias (LeViT-style attention).

Data-parallel over batch across 8 NeuronCores (32 batches/core, 16 pairs).
Cost-model-driven structure:

  - qk/v projections: 3-term hi/lo fp8e4 DoubleRow (x_hi@w_hi + x_lo@w_hi/32
    + x_hi@w_lo) -> 0.75x bf16 PE cost at ~fp32 accuracy.
  - attention bias seeded into PSUM by fp8 DoubleRow identity matmul.
  - S^T and O^T in bf16 (98/98 key chunks; fp8 fails the error budget here).
  - softmax denominator on GPSIMD: chunk-add (f32) + partition_all_reduce,
    freeing the PE of the ones-matmul entirely.
  - normalize: DVE reciprocal + multiply (PSUM evac fused).
  - v-bias/proj-bias folded on host: pb_eff = proj_b + proj_w @ bv.
  - software-pipelined emission: attention stage2 lags stage1 by one block,
    output projection lags by one pair; y-DMA on the Pool queue.
"""

import sys

sys.path.insert(0, "/opt/trn_rl_repo")

from contextlib import ExitStack

import numpy as np
import ml_dtypes

import concourse.bacc as bacc
import concourse.tile as tile
import concourse.mybir as mybir
import concourse.bass_isa as bass_isa
from concourse.bass_utils import run_bass_kernel_spmd

BF16 = ml_dtypes.bfloat16
F8 = ml_dtypes.float8_e4m3fn
DR = mybir.MatmulPerfMode.DoubleRow

B, N_TOK, C = 256, 196, 512
NUM_HEADS, KEY_DIM, D_V = 8, 32, 128
DH = D_V * NUM_HEADS  # 1024
SCALE = KEY_DIM ** (-0.5)

N_CORES = 8
BPC = B // N_CORES  # 32
NPAIR = BPC // 2  # 16
TP = 2 * N_TOK  # 392
CH = 98  # key chunk

WSCALE = 64.0
N_WARM = 110

_cache = {}


def _build_program(npair=NPAIR):
    nc = bacc.Bacc("TRN2", target_bir_lowering=False, debug=False)
    f32 = mybir.dt.float32
    bf16 = mybir.dt.bfloat16
    fp8 = mybir.dt.float8e4

    # ---- DRAM I/O ----
    xT = nc.dram_tensor("xT", [128, npair, 2, 4, 400], fp8, kind="ExternalInput").ap()
    wqk = nc.dram_tensor("wqk", [128, 3, 4, 4, 128], fp8, kind="ExternalInput").ap()
    qkb = nc.dram_tensor("qkb", [4, 128], f32, kind="ExternalInput").ap()
    wv = nc.dram_tensor("wv", [128, 3, 4, DH], fp8, kind="ExternalInput").ap()
    pw = nc.dram_tensor("pw", [128, 4, 2, 4, 128], bf16, kind="ExternalInput").ap()
    pb = nc.dram_tensor("pb", [4, 128], f32, kind="ExternalInput").ap()
    idT = nc.dram_tensor("idT", [CH, 2, 112], fp8, kind="ExternalInput").ap()
    bp = nc.dram_tensor("bp", [CH, 2, NUM_HEADS, TP], fp8, kind="ExternalInput").ap()
    yT = nc.dram_tensor("yT", [128, npair, 4, TP], f32, kind="ExternalOutput").ap()

    with tile.TileContext(nc) as tc, ExitStack() as ctx:
        consts = ctx.enter_context(tc.tile_pool(name="consts", bufs=1))
        xio = ctx.enter_context(tc.tile_pool(name="xio", bufs=3))
        qkp = ctx.enter_context(tc.tile_pool(name="qkp", bufs=3))
        vp = ctx.enter_context(tc.tile_pool(name="vp", bufs=3))
        ep = ctx.enter_context(tc.tile_pool(name="ep", bufs=6))
        dp = ctx.enter_context(tc.tile_pool(name="dp", bufs=9))
        rp = ctx.enter_context(tc.tile_pool(name="rp", bufs=5))
        op = ctx.enter_context(tc.tile_pool(name="op", bufs=2))
        yp = ctx.enter_context(tc.tile_pool(name="yp", bufs=2))
        ps = ctx.enter_context(tc.tile_pool(name="ps", bufs=1, space="PSUM"))

        # ---- constants ----
        wqk_sb = consts.tile([128, 3, 4, 4, 128], fp8)
        nc.sync.dma_start(out=wqk_sb, in_=wqk)
        qkb_sb = consts.tile([128, 4], f32)
        nc.sync.dma_start(out=qkb_sb, in_=qkb.rearrange("k p -> p k"))
        wv_sb = consts.tile([128, 3, 4, DH], fp8)
        nc.scalar.dma_start(out=wv_sb, in_=wv)
        id_sb = consts.tile([CH, 2, 112], fp8)
        nc.scalar.dma_start(out=id_sb, in_=idT)
        bp_sb = consts.tile([CH, 2, NUM_HEADS, TP], fp8)
        nc.scalar.dma_start(out=bp_sb, in_=bp)
        pw_sb = consts.tile([128, 4, 2, 4, 128], bf16)
        nc.scalar.dma_start(out=pw_sb, in_=pw)
        pb_sb = consts.tile([128, 4], f32)
        nc.scalar.dma_start(out=pb_sb, in_=pb.rearrange("k p -> p k"))
        wm_sb = consts.tile([128, 128], bf16)
        nc.gpsimd.memset(wm_sb, 1.0)

        # PE warm-up: p-state ramps to full rate after ~3us of continuous busy.
        warm_ps = ps.tile([128, 512], f32, tag="ps1", bufs=2, name="warm_ps")
        for w in range(N_WARM):
            nc.tensor.matmul(
                warm_ps[:, :128], lhsT=wm_sb, rhs=wm_sb, start=True, stop=True
            )

        def emit_qkv(pair):
            xp = xio.tile([128, 2, 4, 400], fp8, name="xp")
            nc.sync.dma_start(out=xp, in_=xT[:, pair])

            # ---- qk projection: 3-term fp8 DR ----
            qk_sb = qkp.tile([128, 5, 400], bf16, name="qk_sb")
            terms = [(0, 0), (1, 1), (0, 2)]  # (x hi/lo, w hi/hi32/lo)
            for jt in range(4):
                qk_ps = ps.tile([128, TP], f32, tag="ps1", bufs=2, name="qk_ps", padded_shape=(..., 512))
                n = 0
                for xs, wsd in terms:
                    for t in range(2):
                        nc.tensor.matmul(
                            qk_ps,
                            lhsT=wqk_sb[:, wsd, 2 * t : 2 * t + 2, jt, :],
                            rhs=xp[:, xs, 2 * t : 2 * t + 2, :TP],
                            start=(n == 0),
                            stop=(n == 5),
                            perf_mode=DR,
                        )
                        n += 1
                nc.vector.tensor_scalar(
                    out=qk_sb[:, jt, :TP],
                    in0=qk_ps,
                    scalar1=1.0 / WSCALE,
                    scalar2=qkb_sb[:, jt : jt + 1],
                    op0=mybir.AluOpType.mult,
                    op1=mybir.AluOpType.add,
                )

            # ---- v projection: 3-term fp8 DR ----
            v_sb = vp.tile([CH, 4, DH], bf16, name="v_sb")
            for cc in range(4):
                tok0 = CH * cc
                for f in range(2):
                    v_ps = ps.tile([CH, 512], f32, tag="ps1", bufs=2, name="v_ps")
                    n = 0
                    for xs, wsd in terms:
                        for t in range(2):
                            nc.tensor.matmul(
                                v_ps,
                                lhsT=xp[:, xs, 2 * t : 2 * t + 2, tok0 : tok0 + CH],
                                rhs=wv_sb[:, wsd, 2 * t : 2 * t + 2, 512 * f : 512 * (f + 1)],
                                start=(n == 0),
                                stop=(n == 5),
                                perf_mode=DR,
                            )
                            n += 1
                    vdst = v_sb[:, cc, 512 * f : 512 * (f + 1)]
                    nc.vector.tensor_scalar_mul(out=vdst, in0=v_ps, scalar1=1.0 / WSCALE)
            return qk_sb, v_sb

        eb_count = [0]

        def attn_stage1(qk_sb, i, hp):
            t0 = N_TOK * i
            s_ps = ps.tile([CH, 2, 2, N_TOK], f32, tag="psS", bufs=2, name="s_ps", padded_shape=(..., 256))
            for hh in range(2):
                h = 2 * hp + hh
                nc.tensor.matmul(
                    s_ps[:, hh],
                    lhsT=id_sb[:, :, :CH],
                    rhs=bp_sb[:, :, h, :],
                    start=True,
                    stop=False,
                    perf_mode=DR,
                )
                g = h % 4
                jq = h // 4
                jk = 2 + h // 4
                p0 = 32 * g
                q_rhs = qk_sb[p0 : p0 + 32, jq, t0 : t0 + N_TOK]
                for cc in range(2):
                    k_lhs = qk_sb[p0 : p0 + 32, jk, t0 + CH * cc : t0 + CH * (cc + 1)]
                    nc.tensor.matmul(
                        s_ps[:, hh, cc, :],
                        lhsT=k_lhs,
                        rhs=q_rhs,
                        start=False,
                        stop=True,
                        skip_group_check=True,
                        tile_position=(p0, 0),
                    )
            # exp for 2 heads x 2 chunks in one instruction
            e_sb = ep.tile([128, 2, 2, 208], bf16, name="e_sb")
            if eb_count[0] < 6:
                # zero partitions 96..127 on every ring slot (ep bufs=6) so the
                # 128-channel partition reduction reads zeros there; later
                # generations reuse the same bytes untouched.
                nc.vector.memset(e_sb[96:, :, :, :], 0.0)
                eb_count[0] += 1
            for hh in range(2):
                nc.scalar.activation(
                    out=e_sb[:CH, hh, :, :N_TOK],
                    in_=s_ps[:, hh],
                    func=mybir.ActivationFunctionType.Exp,
                    scale=SCALE,
                )
            # denominator on gpsimd: chunk-add then partition all-reduce
            ee_sb = dp.tile([128, 2, N_TOK], f32, name="ee_sb")
            nc.gpsimd.tensor_add(
                out=ee_sb, in0=e_sb[:, :, 0, :N_TOK], in1=e_sb[:, :, 1, :N_TOK]
            )
            d_sb = dp.tile([128, 2, N_TOK], f32, name="d_sb")
            nc.gpsimd.partition_all_reduce(
                out_ap=d_sb, in_ap=ee_sb, channels=128, reduce_op=bass_isa.ReduceOp.add
            )
            return e_sb, d_sb

        def attn_stage2(v_sb, ot_sb, e_sb, d_sb, i, hp):
            t0 = N_TOK * i
            o_ps = ps.tile([128, 2, N_TOK], f32, tag="psOD", bufs=2, name="o_ps", padded_shape=(..., 256))
            for hh in range(2):
                h = 2 * hp + hh
                for cc in range(2):
                    nc.tensor.matmul(
                        o_ps[:, hh, :],
                        lhsT=v_sb[:, 2 * i + cc, 128 * h : 128 * (h + 1)],
                        rhs=e_sb[:CH, hh, cc, :N_TOK],
                        start=(cc == 0),
                        stop=(cc == 1),
                    )
            rec_sb = rp.tile([128, 2, N_TOK], f32, name="rec_sb")
            nc.vector.reciprocal_approx_fast(out=rec_sb, in_=d_sb)
            nc.vector.tensor_mul(
                out=ot_sb[:, 2 * hp : 2 * hp + 2, t0 : t0 + N_TOK],
                in0=o_ps,
                in1=rec_sb,
            )

        def emit_attn(qk_sb, v_sb):
            ot_sb = op.tile([128, NUM_HEADS, 400], bf16, name="ot_sb")
            blocks = [(i, hp) for i in range(2) for hp in range(NUM_HEADS // 2)]
            pend = []
            for i, hp in blocks:
                e_sb, d_sb = attn_stage1(qk_sb, i, hp)
                pend.append((e_sb, d_sb, i, hp))
                if len(pend) > 2:
                    attn_stage2(v_sb, ot_sb, *pend.pop(0))
            return ot_sb, pend

        def emit_proj(pair, ot_sb, last=False):
            y_sb = yp.tile([128, 4, TP], f32, name="y_sb")
            for ct in range(4):
                p_ps = ps.tile([128, TP], f32, tag="ps1", bufs=2, name="p_ps", padded_shape=(..., 512))
                for jc in range(8):
                    nc.tensor.matmul(
                        p_ps,
                        lhsT=pw_sb[:, jc // 2, jc % 2, ct, :],
                        rhs=ot_sb[:, jc, :TP],
                        start=(jc == 0),
                        stop=(jc == 7),
                    )
                nc.scalar.activation(
                    out=y_sb[:, ct, :],
                    in_=p_ps,
                    func=mybir.ActivationFunctionType.Identity,
                    bias=pb_sb[:, ct : ct + 1],
                )
                if last and ct == 1:
                    nc.scalar.dma_start(out=yT[:, pair, :2], in_=y_sb[:, :2])
            if last:
                nc.scalar.dma_start(out=yT[:, pair, 2:], in_=y_sb[:, 2:])
            else:
                nc.scalar.dma_start(out=yT[:, pair], in_=y_sb)

        prev = None
        for pair in range(npair):
            qk_sb, v_sb = emit_qkv(pair)
            if prev is not None:
                p_pair, p_ot, p_pend, p_v = prev
                for p in p_pend:
                    attn_stage2(p_v, p_ot, *p)
                emit_proj(p_pair, p_ot)
            ot_sb, pend = emit_attn(qk_sb, v_sb)
            prev = (pair, ot_sb, pend, v_sb)
        p_pair, p_ot, p_pend, p_v = prev
        for p in p_pend:
            attn_stage2(p_v, p_ot, *p)
        emit_proj(p_pair, p_ot, last=True)

    nc.compile()
    return nc


def _prep_weights(qkv_w, qkv_b, proj_w, proj_b, attention_biases, bias_idxs):
    perm_qk = np.empty(512, dtype=np.int64)
    for jp in range(512):
        jt, r = divmod(jp, 128)
        g, d = divmod(r, 32)
        if jt < 2:
            perm_qk[jp] = (jt * 4 + g) * 192 + d
        else:
            perm_qk[jp] = ((jt - 2) * 4 + g) * 192 + 32 + d
    perm_v = np.empty(DH, dtype=np.int64)
    for jv in range(DH):
        h, dv = divmod(jv, 32 * 4)
        perm_v[jv] = h * 192 + 64 + dv

    def hi_lo(w):  # w: [rows, 512] scaled; -> hi, hi/32, lo as fp8 f32-arrays
        ws = w * WSCALE
        hi = ws.astype(np.float32).astype(F8)
        hif = hi.astype(np.float32)
        hi32 = (hif / 32.0).astype(F8)
        lo = (ws - hif).astype(np.float32).astype(F8)
        return hi, hi32, lo

    wq = qkv_w[perm_qk]  # [512 j', 512 c]
    parts = hi_lo(wq)
    wqk_ = np.zeros((128, 3, 4, 4, 128), dtype=F8)
    for s, p in enumerate(parts):
        # p [j', c] -> [pp, kc, jt, m]
        wqk_[:, s] = np.ascontiguousarray(
            p.reshape(4, 128, 4, 128).transpose(3, 2, 0, 1)
        )
    qkb_ = np.ascontiguousarray(qkv_b[perm_qk].reshape(4, 128)).astype(np.float32)

    wvm = qkv_w[perm_v]  # [1024 j, 512 c]
    parts = hi_lo(wvm)
    wv_ = np.zeros((128, 3, 4, DH), dtype=F8)
    for s, p in enumerate(parts):
        wv_[:, s] = np.ascontiguousarray(p.reshape(DH, 4, 128).transpose(2, 1, 0))

    pw_ = np.ascontiguousarray(
        proj_w.reshape(4, 128, 8, 128).transpose(3, 2, 0, 1).reshape(128, 4, 2, 4, 128)
    ).astype(BF16)
    bv = qkv_b[perm_v]
    pb_eff = proj_b + proj_w @ bv
    pb_ = np.ascontiguousarray(pb_eff.reshape(4, 128)).astype(np.float32)

    bias_full = attention_biases[:, bias_idxs]  # [H, query, key]
    bkq = np.transpose(bias_full, (0, 2, 1)).astype(np.float64)  # [H, key, query]
    bseed = bkq / SCALE
    bs = bseed.reshape(NUM_HEADS, 2, CH, N_TOK).transpose(2, 0, 1, 3)  # [98, H, 2, 196]
    bp_ = np.zeros((CH, 2, NUM_HEADS, TP), dtype=np.float64)
    bp_[:, 0] = bs.reshape(CH, NUM_HEADS, TP)
    bp_ = bp_.astype(F8)
    id_ = np.zeros((CH, 2, 112), dtype=F8)
    id_[:, 0, :CH][np.arange(CH), np.arange(CH)] = 1.0

    return dict(wqk=wqk_, qkb=qkb_, wv=wv_, pw=pw_, pb=pb_, bp=bp_, idT=id_)


def _prep_x_core(x_core):
    npair = x_core.shape[0] // 2
    xt = x_core.reshape(npair, TP, 4, 128).transpose(3, 0, 2, 1)  # [128, np, 4, 392]
    out = np.zeros((128, npair, 2, 4, 400), dtype=F8)
    hi = xt.astype(np.float32).astype(F8)
    out[:, :, 0, :, :TP] = hi
    out[:, :, 1, :, :TP] = ((xt - hi.astype(np.float32)) * 32.0).astype(F8)
    return out


def _unshard_y(y_core, npair=NPAIR):
    y = np.ascontiguousarray(np.transpose(y_core, (1, 3, 2, 0)))
    return y.reshape(npair * 2, N_TOK, C)


def kernel(x, qkv_w, qkv_b, proj_w, proj_b, attention_biases, bias_idxs):
    x = np.asarray(x, dtype=np.float32)
    qkv_w = np.asarray(qkv_w, dtype=np.float32)
    qkv_b = np.asarray(qkv_b, dtype=np.float32)
    proj_w = np.asarray(proj_w, dtype=np.float32)
    proj_b = np.asarray(proj_b, dtype=np.float32)
    attention_biases = np.asarray(attention_biases, dtype=np.float32)
    bias_idxs = np.asarray(bias_idxs)

    if "nc" not in _cache:
        _cache["nc"] = _build_program()
    nc = _cache["nc"]

    wmap = _prep_weights(qkv_w, qkv_b, proj_w, proj_b, attention_biases, bias_idxs)
    in_maps = []
    for core in range(N_CORES):
        m = dict(wmap)
        m["xT"] = _prep_x_core(x[core * BPC : (core + 1) * BPC])
        in_maps.append(m)

    import os

    guard = {}
    try:
        from antenv import axon_hooks  # noqa: F401
    except ImportError:
        if os.environ.get("BASS_TRACE") and not os.environ.get("BASS_NEVER_TRACE"):
            guard["BASS_NEVER_TRACE"] = True
            os.environ["BASS_NEVER_TRACE"] = "1"
    try:
        res = run_bass_kernel_spmd(nc, in_maps, list(range(N_CORES)))
    finally:
        if guard:
            os.environ.pop("BASS_NEVER_TRACE", None)
    _cache["last_res"] = res
    out = np.concatenate(
        [_unshard_y(res.results[i]["yT"]) for i in range(N_CORES)], axis=0
    )
    return out.astype(np.float32)


if __name__ == "__main__":
    print("building program...")
    _build_program(npair=2)
    print("ok")



# revision 38
# speedup vs baseline: 1.0431x; 1.0431x over previous
"""Trainium2 Bass kernel v3 for AttentionWithBias (LeViT-style attention).

Data-parallel over batch across 8 NeuronCores (32 batches/core, 16 pairs).
Cost-model-driven structure:

  - qk/v projections: 3-term hi/lo fp8e4 DoubleRow (x_hi@w_hi + x_lo@w_hi/32
    + x_hi@w_lo) -> 0.75x bf16 PE cost at ~fp32 accuracy.
  - attention bias seeded into PSUM by fp8 DoubleRow identity matmul.
  - S^T and O^T in bf16 (98/98 key chunks; fp8 fails the error budget here).
  - softmax denominator on GPSIMD: chunk-add (f32) + partition_all_reduce,
    freeing the PE of the ones-matmul entirely.
  - normalize: DVE reciprocal + multiply (PSUM evac fused).
  - v-bias/proj-bias folded on host: pb_eff = proj_b + proj_w @ bv.
  - software-pipelined emission: attention stage2 lags stage1 by one block,
    output projection lags by one pair; y-DMA on the Pool queue.
"""

import sys

sys.path.insert(0, "/opt/trn_rl_repo")

from contextlib import ExitStack

import numpy as np
import ml_dtypes

import concourse.bacc as bacc
import concourse.tile as tile
import concourse.mybir as mybir
import concourse.bass_isa as bass_isa
from concourse.bass_utils import run_bass_kernel_spmd

BF16 = ml_dtypes.bfloat16
F8 = ml_dtypes.float8_e4m3fn
DR = mybir.MatmulPerfMode.DoubleRow

B, N_TOK, C = 256, 196, 512
NUM_HEADS, KEY_DIM, D_V = 8, 32, 128
DH = D_V * NUM_HEADS  # 1024
SCALE = KEY_DIM ** (-0.5)

N_CORES = 8
BPC = B // N_CORES  # 32
NPAIR = BPC // 2  # 16
TP = 2 * N_TOK  # 392
CH = 98  # key chunk

WSCALE = 64.0
N_WARM = 56
LAG = 5

_cache = {}


def _build_program(npair=NPAIR):
    nc = bacc.Bacc("TRN2", target_bir_lowering=False, debug=False)
    f32 = mybir.dt.float32
    f16 = mybir.dt.float16
    bf16 = mybir.dt.bfloat16
    fp8 = mybir.dt.float8e4

    # ---- DRAM I/O ----
    xT = nc.dram_tensor("xT", [128, npair, 2, 4, 400], fp8, kind="ExternalInput").ap()
    wqk = nc.dram_tensor("wqk", [128, 3, 4, 4, 128], fp8, kind="ExternalInput").ap()
    qkb = nc.dram_tensor("qkb", [4, 128], f32, kind="ExternalInput").ap()
    wv = nc.dram_tensor("wv", [128, 3, 4, DH], fp8, kind="ExternalInput").ap()
    pw = nc.dram_tensor("pw", [128, 4, 2, 4, 128], bf16, kind="ExternalInput").ap()
    pb = nc.dram_tensor("pb", [4, 128], f32, kind="ExternalInput").ap()
    idT = nc.dram_tensor("idT", [CH, 2, 112], fp8, kind="ExternalInput").ap()
    bp = nc.dram_tensor("bp", [CH, 2, NUM_HEADS, TP], fp8, kind="ExternalInput").ap()
    yT = nc.dram_tensor("yT", [128, npair, 4, TP], f32, kind="ExternalOutput").ap()

    with tile.TileContext(nc) as tc, ExitStack() as ctx:
        consts = ctx.enter_context(tc.tile_pool(name="consts", bufs=1))
        xio = ctx.enter_context(tc.tile_pool(name="xio", bufs=3))
        qkp = ctx.enter_context(tc.tile_pool(name="qkp", bufs=3))
        vp = ctx.enter_context(tc.tile_pool(name="vp", bufs=3))
        ep = ctx.enter_context(tc.tile_pool(name="ep", bufs=7))
        dp = ctx.enter_context(tc.tile_pool(name="dp", bufs=11))
        rp = ctx.enter_context(tc.tile_pool(name="rp", bufs=5))
        op = ctx.enter_context(tc.tile_pool(name="op", bufs=2))
        yp = ctx.enter_context(tc.tile_pool(name="yp", bufs=2))
        ps = ctx.enter_context(tc.tile_pool(name="ps", bufs=1, space="PSUM"))

        # ---- constants ----
        # The cost model runs all DMA transfers through ONE serial pipe, in
        # HWDGE arrival order. Everything goes on the SP queue in exactly the
        # order the PE will need it: wqk -> x(pair0) -> qkb -> wv(hi,lo) ->
        # wv(hi32) -> x(pair1) -> bias tables -> proj weights.
        wqk_sb = consts.tile([128, 3, 4, 4, 128], fp8)
        nc.sync.dma_start(out=wqk_sb[:, :2], in_=wqk[:, :2])
        xp0_sb = xio.tile([128, 2, 4, 400], fp8, name="xp")
        nc.sync.dma_start(out=xp0_sb, in_=xT[:, 0])
        qkb_sb = consts.tile([128, 4], f32)
        nc.sync.dma_start(out=qkb_sb, in_=qkb.rearrange("k p -> p k"))
        wv_sb = consts.tile([128, 3, 4, DH], fp8)
        nc.sync.dma_start(out=wv_sb[:, :2], in_=wv[:, :2])
        xp1_sb = xio.tile([128, 2, 4, 400], fp8, name="xp")
        nc.sync.dma_start(out=xp1_sb, in_=xT[:, 1])
        # hi32 = hi * 2^-5 is derived on-chip (exact: fp8 -> f32 -> scale ->
        # fp8 round matches the host computation), keeping the serial DMA
        # pipe ~3.5us shorter at the critical kernel start. The wv derives
        # are emitted inside emit_qkv(0), after the pair-0 qk evacuations,
        # to avoid head-of-line blocking DVE's queue.
        nc.vector.tensor_scalar_mul(
            out=wqk_sb[:, 2], in0=wqk_sb[:, 0], scalar1=1.0 / 32.0
        )
        bp_sb = consts.tile([CH, 2, NUM_HEADS, TP], fp8)
        nc.sync.dma_start(out=bp_sb, in_=bp)
        id_sb = consts.tile([CH, 2, 112], fp8)
        nc.sync.dma_start(out=id_sb, in_=idT)
        pw_sb = consts.tile([128, 4, 2, 4, 128], bf16)
        nc.sync.dma_start(out=pw_sb, in_=pw)
        pb_sb = consts.tile([128, 4], f32)
        nc.sync.dma_start(out=pb_sb, in_=pb.rearrange("k p -> p k"))
        wm_sb = consts.tile([128, 128], bf16)
        nc.vector.memset(wm_sb, 1.0)

        # PE warm-up: p-state ramps to full rate after ~3us of continuous busy.
        def emit_fill(n):
            warm_ps = ps.tile([128, 512], f32, tag="ps1", bufs=3, name="warm_ps")
            for w in range(n):
                nc.tensor.matmul(
                    warm_ps[:, :128], lhsT=wm_sb, rhs=wm_sb, start=True, stop=True
                )

        emit_fill(N_WARM)

        def emit_qkv(pair, mid=None, xp=None, fill_after_qk=0):
            if xp is None:
                xp = xio.tile([128, 2, 4, 400], fp8, name="xp")
                nc.sync.dma_start(out=xp, in_=xT[:, pair])

            # ---- qk projection: 3-term fp8 DR ----
            qk_sb = qkp.tile([128, 5, 400], bf16, name="qk_sb")
            # (x hi/lo, w hi/lo/hi32); the hi32 term last so the derived wv
            # hi32 slice isn't needed until each gen's end
            terms = [(0, 0), (0, 1), (1, 2)]
            for jt in range(4):
                qk_ps = ps.tile([128, TP], f32, tag="ps1", bufs=3, name="qk_ps", padded_shape=(..., 512))
                n = 0
                for xs, wsd in terms:
                    for t in range(2):
                        nc.tensor.matmul(
                            qk_ps,
                            lhsT=wqk_sb[:, wsd, 2 * t : 2 * t + 2, jt, :],
                            rhs=xp[:, xs, 2 * t : 2 * t + 2, :TP],
                            start=(n == 0),
                            stop=(n == 5),
                            perf_mode=DR,
                        )
                        n += 1
                nc.vector.tensor_scalar(
                    out=qk_sb[:, jt, :TP],
                    in0=qk_ps,
                    scalar1=1.0 / WSCALE,
                    scalar2=qkb_sb[:, jt : jt + 1],
                    op0=mybir.AluOpType.mult,
                    op1=mybir.AluOpType.add,
                )

            if mid is not None:
                # the previous pair's stage2 backlog slots in here, so its
                # DVE norms land ahead of the v evacuations in DVE's queue
                mid()
            if pair == 0:
                nc.vector.tensor_scalar_mul(
                    out=wv_sb[:, 2, :, :512], in0=wv_sb[:, 0, :, :512], scalar1=1.0 / 32.0
                )
                nc.vector.tensor_scalar_mul(
                    out=wv_sb[:, 2, :, 512:], in0=wv_sb[:, 0, :, 512:], scalar1=1.0 / 32.0
                )
            if fill_after_qk:
                # pair 0 only: the wv DMA is still in flight when the qk
                # matmuls finish; keep the PE p-state warm until it lands
                emit_fill(fill_after_qk)

            # ---- v projection: 3-term fp8 DR ----
            v_sb = vp.tile([CH, 4, DH], bf16, name="v_sb")
            for cc in range(4):
                tok0 = CH * cc
                for f in range(2):
                    v_ps = ps.tile([CH, 512], f32, tag="ps1", bufs=3, name="v_ps")
                    n = 0
                    for xs, wsd in terms:
                        for t in range(2):
                            nc.tensor.matmul(
                                v_ps,
                                lhsT=xp[:, xs, 2 * t : 2 * t + 2, tok0 : tok0 + CH],
                                rhs=wv_sb[:, wsd, 2 * t : 2 * t + 2, 512 * f : 512 * (f + 1)],
                                start=(n == 0),
                                stop=(n == 5),
                                perf_mode=DR,
                            )
                            n += 1
                    vdst = v_sb[:, cc, 512 * f : 512 * (f + 1)]
                    nc.vector.tensor_scalar_mul(out=vdst, in0=v_ps, scalar1=1.0 / WSCALE)
            return qk_sb, v_sb

        eb_count = [0]

        def attn_stage1(qk_sb, i, hp):
            t0 = N_TOK * i
            e_sb = ep.tile([128, 2, 2, 208], bf16, name="e_sb")
            if eb_count[0] < 7:
                # zero partitions 96..127 on every ring slot (ep bufs=7) so the
                # 128-channel partition reduction reads zeros there; later
                # generations reuse the same bytes untouched. Pool is idle in
                # the ramp-up phase, so these don't block the evac engines.
                nc.gpsimd.memset(e_sb[96:, :, :, :], 0.0)
                eb_count[0] += 1
            for hh in range(2):
                h = 2 * hp + hh
                # per-head 1-bank PSUM tile (vs 2-bank for the head pair):
                # frees a PSUM bank so the shared ps1 ring can triple-buffer
                s_ps = ps.tile([CH, 2, N_TOK], f32, tag="psS", bufs=3, name="s_ps", padded_shape=(..., 208))
                nc.tensor.matmul(
                    s_ps,
                    lhsT=id_sb[:, :, :CH],
                    rhs=bp_sb[:, :, h, :],
                    start=True,
                    stop=False,
                    perf_mode=DR,
                )
                g = h % 4
                jq = h // 4
                jk = 2 + h // 4
                p0 = 32 * g
                q_rhs = qk_sb[p0 : p0 + 32, jq, t0 : t0 + N_TOK]
                for cc in range(2):
                    k_lhs = qk_sb[p0 : p0 + 32, jk, t0 + CH * cc : t0 + CH * (cc + 1)]
                    nc.tensor.matmul(
                        s_ps[:, cc, :],
                        lhsT=k_lhs,
                        rhs=q_rhs,
                        start=False,
                        stop=True,
                        skip_group_check=True,
                        tile_position=(p0, 0),
                    )
                nc.scalar.activation(
                    out=e_sb[:CH, hh, :, :N_TOK],
                    in_=s_ps,
                    func=mybir.ActivationFunctionType.Exp,
                    scale=SCALE,
                )
            # denominator: chunk-add on DVE (all-2-byte operands hit the 2x
            # path; fp16 keeps ~11 mantissa bits and cannot overflow here),
            # then partition all-reduce on Pool (f32 accumulate)
            ee_sb = dp.tile([128, 2, N_TOK], f16, name="ee_sb")
            nc.vector.tensor_add(
                out=ee_sb, in0=e_sb[:, :, 0, :N_TOK], in1=e_sb[:, :, 1, :N_TOK]
            )
            d_sb = dp.tile([128, 2, N_TOK], f32, name="d_sb")
            nc.gpsimd.partition_all_reduce(
                out_ap=d_sb, in_ap=ee_sb, channels=128, reduce_op=bass_isa.ReduceOp.add
            )
            return e_sb, d_sb

        def attn_stage2(v_sb, ot_sb, e_sb, d_sb, i, hp):
            t0 = N_TOK * i
            o_ps = ps.tile([128, 2, N_TOK], f32, tag="psOD", bufs=2, name="o_ps", padded_shape=(..., 256))
            for hh in range(2):
                h = 2 * hp + hh
                for cc in range(2):
                    nc.tensor.matmul(
                        o_ps[:, hh, :],
                        lhsT=v_sb[:, 2 * i + cc, 128 * h : 128 * (h + 1)],
                        rhs=e_sb[:CH, hh, cc, :N_TOK],
                        start=(cc == 0),
                        stop=(cc == 1),
                    )
            rec_sb = rp.tile([128, 2, N_TOK], f32, name="rec_sb")
            nc.vector.reciprocal_approx_fast(out=rec_sb, in_=d_sb)
            nc.vector.tensor_mul(
                out=ot_sb[:, 2 * hp : 2 * hp + 2, t0 : t0 + N_TOK],
                in0=o_ps,
                in1=rec_sb,
            )

        def emit_attn(qk_sb, v_sb):
            ot_sb = op.tile([128, NUM_HEADS, 400], bf16, name="ot_sb")
            blocks = [(i, hp) for i in range(2) for hp in range(NUM_HEADS // 2)]
            pend = []
            for i, hp in blocks:
                e_sb, d_sb = attn_stage1(qk_sb, i, hp)
                pend.append((e_sb, d_sb, i, hp))
                if len(pend) > LAG - 1:
                    attn_stage2(v_sb, ot_sb, *pend.pop(0))
            return ot_sb, pend

        def emit_proj(pair, ot_sb, last=False):
            y_sb = yp.tile([128, 4, TP], f32, name="y_sb")
            for ct in range(4):
                p_ps = ps.tile([128, TP], f32, tag="ps1", bufs=3, name="p_ps", padded_shape=(..., 512))
                for jc in range(8):
                    nc.tensor.matmul(
                        p_ps,
                        lhsT=pw_sb[:, jc // 2, jc % 2, ct, :],
                        rhs=ot_sb[:, jc, :TP],
                        start=(jc == 0),
                        stop=(jc == 7),
                    )
                nc.scalar.activation(
                    out=y_sb[:, ct, :],
                    in_=p_ps,
                    func=mybir.ActivationFunctionType.Identity,
                    bias=pb_sb[:, ct : ct + 1],
                )
                if last:
                    # drain the tail: ship each ct slice as soon as it lands
                    nc.sync.dma_start(
                        out=yT[:, pair, ct : ct + 1], in_=y_sb[:, ct : ct + 1]
                    )
            if not last:
                nc.sync.dma_start(out=yT[:, pair], in_=y_sb)

        qkv = {
            0: emit_qkv(0, xp=xp0_sb, fill_after_qk=42),
            1: emit_qkv(1, xp=xp1_sb),
        }
        for pair in range(npair):
            qk_sb, v_sb = qkv.pop(pair)
            ot_sb, pend = emit_attn(qk_sb, v_sb)

            def backlog(pend=pend, v_sb=v_sb, ot_sb=ot_sb):
                for p in pend:
                    attn_stage2(v_sb, ot_sb, *p)

            if pair + 2 < npair:
                qkv[pair + 2] = emit_qkv(pair + 2, mid=backlog)
            else:
                backlog()
                # no qkv to overlap at the tail: pad so the last projections
                # don't idle the PE (and drop its p-state) while the final
                # d-chain completes
                emit_fill(20)
            emit_proj(pair, ot_sb, last=(pair == npair - 1))

    nc.compile()
    return nc


def _prep_weights(qkv_w, qkv_b, proj_w, proj_b, attention_biases, bias_idxs):
    perm_qk = np.empty(512, dtype=np.int64)
    for jp in range(512):
        jt, r = divmod(jp, 128)
        g, d = divmod(r, 32)
        if jt < 2:
            perm_qk[jp] = (jt * 4 + g) * 192 + d
        else:
            perm_qk[jp] = ((jt - 2) * 4 + g) * 192 + 32 + d
    perm_v = np.empty(DH, dtype=np.int64)
    for jv in range(DH):
        h, dv = divmod(jv, 32 * 4)
        perm_v[jv] = h * 192 + 64 + dv

    def hi_lo(w):  # w: [rows, 512] scaled; -> hi, lo, hi/32 as fp8 f32-arrays
        # slice order (hi, lo, hi32): the kernel DMAs slices 0-1 and derives
        # slice 2 = slice0 * 2^-5 on-chip, so hi32 must be the LAST slice
        ws = w * WSCALE
        hi = ws.astype(np.float32).astype(F8)
        hif = hi.astype(np.float32)
        hi32 = (hif / 32.0).astype(F8)
        lo = (ws - hif).astype(np.float32).astype(F8)
        return hi, lo, hi32

    wq = qkv_w[perm_qk]  # [512 j', 512 c]
    parts = hi_lo(wq)
    wqk_ = np.zeros((128, 3, 4, 4, 128), dtype=F8)
    for s, p in enumerate(parts):
        # p [j', c] -> [pp, kc, jt, m]
        wqk_[:, s] = np.ascontiguousarray(
            p.reshape(4, 128, 4, 128).transpose(3, 2, 0, 1)
        )
    qkb_ = np.ascontiguousarray(qkv_b[perm_qk].reshape(4, 128)).astype(np.float32)

    wvm = qkv_w[perm_v]  # [1024 j, 512 c]
    parts = hi_lo(wvm)
    wv_ = np.zeros((128, 3, 4, DH), dtype=F8)
    for s, p in enumerate(parts):
        wv_[:, s] = np.ascontiguousarray(p.reshape(DH, 4, 128).transpose(2, 1, 0))

    pw_ = np.ascontiguousarray(
        proj_w.reshape(4, 128, 8, 128).transpose(3, 2, 0, 1).reshape(128, 4, 2, 4, 128)
    ).astype(BF16)
    bv = qkv_b[perm_v]
    pb_eff = proj_b + proj_w @ bv
    pb_ = np.ascontiguousarray(pb_eff.reshape(4, 128)).astype(np.float32)

    bias_full = attention_biases[:, bias_idxs]  # [H, query, key]
    bkq = np.transpose(bias_full, (0, 2, 1)).astype(np.float64)  # [H, key, query]
    bseed = bkq / SCALE
    bs = bseed.reshape(NUM_HEADS, 2, CH, N_TOK).transpose(2, 0, 1, 3)  # [98, H, 2, 196]
    bp_ = np.zeros((CH, 2, NUM_HEADS, TP), dtype=np.float64)
    bp_[:, 0] = bs.reshape(CH, NUM_HEADS, TP)
    bp_ = bp_.astype(F8)
    id_ = np.zeros((CH, 2, 112), dtype=F8)
    id_[:, 0, :CH][np.arange(CH), np.arange(CH)] = 1.0

    return dict(wqk=wqk_, qkb=qkb_, wv=wv_, pw=pw_, pb=pb_, bp=bp_, idT=id_)


def _prep_x_core(x_core):
    npair = x_core.shape[0] // 2
    xt = x_core.reshape(npair, TP, 4, 128).transpose(3, 0, 2, 1)  # [128, np, 4, 392]
    out = np.zeros((128, npair, 2, 4, 400), dtype=F8)
    hi = xt.astype(np.float32).astype(F8)
    out[:, :, 0, :, :TP] = hi
    out[:, :, 1, :, :TP] = ((xt - hi.astype(np.float32)) * 32.0).astype(F8)
    return out


def _unshard_y(y_core, npair=NPAIR):
    y = np.ascontiguousarray(np.transpose(y_core, (1, 3, 2, 0)))
    return y.reshape(npair * 2, N_TOK, C)


def kernel(x, qkv_w, qkv_b, proj_w, proj_b, attention_biases, bias_idxs):
    x = np.asarray(x, dtype=np.float32)
    qkv_w = np.asarray(qkv_w, dtype=np.float32)
    qkv_b = np.asarray(qkv_b, dtype=np.float32)
    proj_w = np.asarray(proj_w, dtype=np.float32)
    proj_b = np.asarray(proj_b, dtype=np.float32)
    attention_biases = np.asarray(attention_biases, dtype=np.float32)
    bias_idxs = np.asarray(bias_idxs)

    if "nc" not in _cache:
        _cache["nc"] = _build_program()
    nc = _cache["nc"]

    wmap = _prep_weights(qkv_w, qkv_b, proj_w, proj_b, attention_biases, bias_idxs)
    in_maps = []
    for core in range(N_CORES):
        m = dict(wmap)
        m["xT"] = _prep_x_core(x[core * BPC : (core + 1) * BPC])
        in_maps.append(m)

    import os

    guard = {}
    try:
        from antenv import axon_hooks  # noqa: F401
    except ImportError:
        if os.environ.get("BASS_TRACE") and not os.environ.get("BASS_NEVER_TRACE"):
            guard["BASS_NEVER_TRACE"] = True
            os.environ["BASS_NEVER_TRACE"] = "1"
    try:
        res = run_bass_kernel_spmd(nc, in_maps, list(range(N_CORES)))
    finally:
        if guard:
            os.environ.pop("BASS_NEVER_TRACE", None)
    _cache["last_res"] = res
    out = np.concatenate(
        [_unshard_y(res.results[i]["yT"]) for i in range(N_CORES)], axis=0
    )
    return out.astype(np.float32)


if __name__ == "__main__":
    print("building program...")
    _build_program(npair=2)
    print("ok")



# revision 42
# speedup vs baseline: 1.0439x; 1.0008x over previous
"""Trainium2 Bass kernel v4 for AttentionWithBias (LeViT-style attention).

Data-parallel over batch across 8 NeuronCores (32 batches/core, 16 pairs).
Cost-model-driven structure (342.7us -> 328.2us vs the v3 baseline):

  - qk/v projections: 3-term hi/lo fp8e4 DoubleRow (x_hi@w_hi + x_hi@w_lo
    + x_lo@w_hi/32) -> 0.75x bf16 PE cost at ~fp32 accuracy. The w_hi/32
    slice is derived on-chip (exact) to shorten the serial start DMA pipe.
  - attention bias seeded into PSUM by fp8 DoubleRow identity matmul.
  - S^T and O^T in bf16 (98/98 key chunks; fp8 fails the error budget here).
  - per-head 1-bank S PSUM tiles (3 bufs) free a bank so the shared
    qkv/proj PSUM ring can triple-buffer -> no evacuation stalls.
  - softmax denominator: fp16 chunk-add on DVE (2x path, exact here) +
    partition_all_reduce on GPSIMD; normalize: DVE reciprocal + multiply.
    (DVE ALU `divide` would fuse these but the executor rejects it.)
  - v-bias/proj-bias folded on host: pb_eff = proj_b + proj_w @ bv.
  - software-pipelined emission: stage2 lags stage1 by LAG blocks; the
    stage2 backlog is spliced between the next pair's qk and v sections;
    qkv is prefetched two pairs deep at the start.
  - all DMAs on the SP queue in exact need-order (the cost model runs one
    serial DMA pipe); PE warm-up and fill matmuls bridge DMA waits so the
    p-state never drops; the last pair ships y per-ct to shorten the drain.
"""

import sys

sys.path.insert(0, "/opt/trn_rl_repo")

from contextlib import ExitStack

import numpy as np
import ml_dtypes

import concourse.bacc as bacc
import concourse.tile as tile
import concourse.mybir as mybir
import concourse.bass_isa as bass_isa
from concourse.bass_utils import run_bass_kernel_spmd

BF16 = ml_dtypes.bfloat16
F8 = ml_dtypes.float8_e4m3fn
DR = mybir.MatmulPerfMode.DoubleRow

B, N_TOK, C = 256, 196, 512
NUM_HEADS, KEY_DIM, D_V = 8, 32, 128
DH = D_V * NUM_HEADS  # 1024
SCALE = KEY_DIM ** (-0.5)

N_CORES = 8
BPC = B // N_CORES  # 32
NPAIR = BPC // 2  # 16
TP = 2 * N_TOK  # 392
CH = 98  # key chunk

WSCALE = 64.0
N_WARM = 56
LAG = 5

_cache = {}


def _build_program(npair=NPAIR):
    nc = bacc.Bacc("TRN2", target_bir_lowering=False, debug=False)
    f32 = mybir.dt.float32
    f16 = mybir.dt.float16
    bf16 = mybir.dt.bfloat16
    fp8 = mybir.dt.float8e4

    # ---- DRAM I/O ----
    xT = nc.dram_tensor("xT", [128, npair, 2, 4, 400], fp8, kind="ExternalInput").ap()
    wqk = nc.dram_tensor("wqk", [128, 3, 4, 4, 128], fp8, kind="ExternalInput").ap()
    qkb = nc.dram_tensor("qkb", [4, 128], f32, kind="ExternalInput").ap()
    wv = nc.dram_tensor("wv", [128, 3, 4, DH], fp8, kind="ExternalInput").ap()
    pw = nc.dram_tensor("pw", [128, 4, 2, 4, 128], bf16, kind="ExternalInput").ap()
    pb = nc.dram_tensor("pb", [4, 128], f32, kind="ExternalInput").ap()
    idT = nc.dram_tensor("idT", [CH, 2, 112], fp8, kind="ExternalInput").ap()
    bp = nc.dram_tensor("bp", [CH, 2, NUM_HEADS, TP], fp8, kind="ExternalInput").ap()
    yT = nc.dram_tensor("yT", [128, npair, 4, TP], f32, kind="ExternalOutput").ap()

    with tile.TileContext(nc) as tc, ExitStack() as ctx:
        consts = ctx.enter_context(tc.tile_pool(name="consts", bufs=1))
        xio = ctx.enter_context(tc.tile_pool(name="xio", bufs=3))
        qkp = ctx.enter_context(tc.tile_pool(name="qkp", bufs=3))
        vp = ctx.enter_context(tc.tile_pool(name="vp", bufs=3))
        ep = ctx.enter_context(tc.tile_pool(name="ep", bufs=7))
        dp = ctx.enter_context(tc.tile_pool(name="dp", bufs=11))
        rp = ctx.enter_context(tc.tile_pool(name="rp", bufs=5))
        op = ctx.enter_context(tc.tile_pool(name="op", bufs=2))
        yp = ctx.enter_context(tc.tile_pool(name="yp", bufs=2))
        ps = ctx.enter_context(tc.tile_pool(name="ps", bufs=1, space="PSUM"))

        # ---- constants ----
        # The cost model runs all DMA transfers through ONE serial pipe, in
        # HWDGE arrival order. Everything goes on the SP queue in exactly the
        # order the PE will need it: wqk -> x(pair0) -> qkb -> wv(hi,lo) ->
        # wv(hi32) -> x(pair1) -> bias tables -> proj weights.
        wqk_sb = consts.tile([128, 3, 4, 4, 128], fp8)
        nc.sync.dma_start(out=wqk_sb[:, :2], in_=wqk[:, :2])
        xp0_sb = xio.tile([128, 2, 4, 400], fp8, name="xp")
        nc.sync.dma_start(out=xp0_sb, in_=xT[:, 0])
        qkb_sb = consts.tile([128, 4], f32)
        nc.sync.dma_start(out=qkb_sb, in_=qkb.rearrange("k p -> p k"))
        wv_sb = consts.tile([128, 3, 4, DH], fp8)
        nc.sync.dma_start(out=wv_sb[:, :2], in_=wv[:, :2])
        xp1_sb = xio.tile([128, 2, 4, 400], fp8, name="xp")
        nc.sync.dma_start(out=xp1_sb, in_=xT[:, 1])
        # hi32 = hi * 2^-5 is derived on-chip (exact: fp8 -> f32 -> scale ->
        # fp8 round matches the host computation), keeping the serial DMA
        # pipe ~3.5us shorter at the critical kernel start. The wv derives
        # are emitted inside emit_qkv(0), after the pair-0 qk evacuations,
        # to avoid head-of-line blocking DVE's queue.
        nc.vector.tensor_scalar_mul(
            out=wqk_sb[:, 2], in0=wqk_sb[:, 0], scalar1=1.0 / 32.0
        )
        bp_sb = consts.tile([CH, 2, NUM_HEADS, TP], fp8)
        nc.sync.dma_start(out=bp_sb, in_=bp)
        id_sb = consts.tile([CH, 2, 112], fp8)
        nc.sync.dma_start(out=id_sb, in_=idT)
        pw_sb = consts.tile([128, 4, 2, 4, 128], bf16)
        nc.sync.dma_start(out=pw_sb, in_=pw)
        pb_sb = consts.tile([128, 4], f32)
        nc.sync.dma_start(out=pb_sb, in_=pb.rearrange("k p -> p k"))
        wm_sb = consts.tile([128, 128], bf16)
        nc.vector.memset(wm_sb, 1.0)

        # PE warm-up: p-state ramps to full rate after ~3us of continuous busy.
        def emit_fill(n):
            warm_ps = ps.tile([128, 512], f32, tag="ps1", bufs=3, name="warm_ps")
            for w in range(n):
                nc.tensor.matmul(
                    warm_ps[:, :128], lhsT=wm_sb, rhs=wm_sb, start=True, stop=True
                )

        emit_fill(N_WARM)

        def emit_qkv(pair, mid=None, xp=None, fill_after_qk=0):
            if xp is None:
                xp = xio.tile([128, 2, 4, 400], fp8, name="xp")
                nc.sync.dma_start(out=xp, in_=xT[:, pair])

            # ---- qk projection: 3-term fp8 DR ----
            qk_sb = qkp.tile([128, 5, 400], bf16, name="qk_sb")
            # (x hi/lo, w hi/lo/hi32); the hi32 term last so the derived wv
            # hi32 slice isn't needed until each gen's end
            terms = [(0, 0), (0, 1), (1, 2)]
            for jt in range(4):
                qk_ps = ps.tile([128, TP], f32, tag="ps1", bufs=3, name="qk_ps", padded_shape=(..., 512))
                n = 0
                for xs, wsd in terms:
                    for t in range(2):
                        nc.tensor.matmul(
                            qk_ps,
                            lhsT=wqk_sb[:, wsd, 2 * t : 2 * t + 2, jt, :],
                            rhs=xp[:, xs, 2 * t : 2 * t + 2, :TP],
                            start=(n == 0),
                            stop=(n == 5),
                            perf_mode=DR,
                        )
                        n += 1
                nc.vector.tensor_scalar(
                    out=qk_sb[:, jt, :TP],
                    in0=qk_ps,
                    scalar1=1.0 / WSCALE,
                    scalar2=qkb_sb[:, jt : jt + 1],
                    op0=mybir.AluOpType.mult,
                    op1=mybir.AluOpType.add,
                )

            if mid is not None:
                # the previous pair's stage2 backlog slots in here, so its
                # DVE norms land ahead of the v evacuations in DVE's queue
                mid()
            if pair == 0:
                nc.vector.tensor_scalar_mul(
                    out=wv_sb[:, 2, :, :512], in0=wv_sb[:, 0, :, :512], scalar1=1.0 / 32.0
                )
                nc.vector.tensor_scalar_mul(
                    out=wv_sb[:, 2, :, 512:], in0=wv_sb[:, 0, :, 512:], scalar1=1.0 / 32.0
                )
            if fill_after_qk:
                # pair 0 only: the wv DMA is still in flight when the qk
                # matmuls finish; keep the PE p-state warm until it lands
                emit_fill(fill_after_qk)

            # ---- v projection: 3-term fp8 DR ----
            v_sb = vp.tile([CH, 4, DH], bf16, name="v_sb")
            for cc in range(4):
                tok0 = CH * cc
                for f in range(2):
                    v_ps = ps.tile([CH, 512], f32, tag="ps1", bufs=3, name="v_ps")
                    n = 0
                    for xs, wsd in terms:
                        for t in range(2):
                            nc.tensor.matmul(
                                v_ps,
                                lhsT=xp[:, xs, 2 * t : 2 * t + 2, tok0 : tok0 + CH],
                                rhs=wv_sb[:, wsd, 2 * t : 2 * t + 2, 512 * f : 512 * (f + 1)],
                                start=(n == 0),
                                stop=(n == 5),
                                perf_mode=DR,
                            )
                            n += 1
                    vdst = v_sb[:, cc, 512 * f : 512 * (f + 1)]
                    nc.vector.tensor_scalar_mul(out=vdst, in0=v_ps, scalar1=1.0 / WSCALE)
            return qk_sb, v_sb

        eb_count = [0]

        def attn_stage1(qk_sb, i, hp):
            t0 = N_TOK * i
            e_sb = ep.tile([128, 2, 2, 208], bf16, name="e_sb")
            if eb_count[0] < 7:
                # zero partitions 96..127 on every ring slot (ep bufs=7) so the
                # 128-channel partition reduction reads zeros there; later
                # generations reuse the same bytes untouched. Pool is idle in
                # the ramp-up phase, so these don't block the evac engines.
                nc.gpsimd.memset(e_sb[96:, :, :, :], 0.0)
                eb_count[0] += 1
            for hh in range(2):
                h = 2 * hp + hh
                # per-head 1-bank PSUM tile (vs 2-bank for the head pair):
                # frees a PSUM bank so the shared ps1 ring can triple-buffer
                s_ps = ps.tile([CH, 2, N_TOK], f32, tag="psS", bufs=3, name="s_ps", padded_shape=(..., 208))
                nc.tensor.matmul(
                    s_ps,
                    lhsT=id_sb[:, :, :CH],
                    rhs=bp_sb[:, :, h, :],
                    start=True,
                    stop=False,
                    perf_mode=DR,
                )
                g = h % 4
                jq = h // 4
                jk = 2 + h // 4
                p0 = 32 * g
                q_rhs = qk_sb[p0 : p0 + 32, jq, t0 : t0 + N_TOK]
                for cc in range(2):
                    k_lhs = qk_sb[p0 : p0 + 32, jk, t0 + CH * cc : t0 + CH * (cc + 1)]
                    nc.tensor.matmul(
                        s_ps[:, cc, :],
                        lhsT=k_lhs,
                        rhs=q_rhs,
                        start=False,
                        stop=True,
                        skip_group_check=True,
                        tile_position=(p0, 0),
                    )
                nc.scalar.activation(
                    out=e_sb[:CH, hh, :, :N_TOK],
                    in_=s_ps,
                    func=mybir.ActivationFunctionType.Exp,
                    scale=SCALE,
                )
            # denominator: chunk-add on DVE (all-2-byte operands hit the 2x
            # path; fp16 keeps ~11 mantissa bits and cannot overflow here),
            # then partition all-reduce on Pool (f32 accumulate)
            ee_sb = dp.tile([128, 2, N_TOK], f16, name="ee_sb")
            nc.vector.tensor_add(
                out=ee_sb, in0=e_sb[:, :, 0, :N_TOK], in1=e_sb[:, :, 1, :N_TOK]
            )
            d_sb = dp.tile([128, 2, N_TOK], f32, name="d_sb")
            nc.gpsimd.partition_all_reduce(
                out_ap=d_sb, in_ap=ee_sb, channels=128, reduce_op=bass_isa.ReduceOp.add
            )
            return e_sb, d_sb

        def attn_stage2(v_sb, ot_sb, e_sb, d_sb, i, hp):
            t0 = N_TOK * i
            o_ps = ps.tile([128, 2, N_TOK], f32, tag="psOD", bufs=2, name="o_ps", padded_shape=(..., 256))
            for hh in range(2):
                h = 2 * hp + hh
                for cc in range(2):
                    nc.tensor.matmul(
                        o_ps[:, hh, :],
                        lhsT=v_sb[:, 2 * i + cc, 128 * h : 128 * (h + 1)],
                        rhs=e_sb[:CH, hh, cc, :N_TOK],
                        start=(cc == 0),
                        stop=(cc == 1),
                    )
            rec_sb = rp.tile([128, 2, N_TOK], f32, name="rec_sb")
            nc.vector.reciprocal_approx_fast(out=rec_sb, in_=d_sb)
            nc.vector.tensor_mul(
                out=ot_sb[:, 2 * hp : 2 * hp + 2, t0 : t0 + N_TOK],
                in0=o_ps,
                in1=rec_sb,
            )

        def emit_attn(qk_sb, v_sb):
            ot_sb = op.tile([128, NUM_HEADS, 400], bf16, name="ot_sb")
            blocks = [(i, hp) for i in range(2) for hp in range(NUM_HEADS // 2)]
            pend = []
            for i, hp in blocks:
                e_sb, d_sb = attn_stage1(qk_sb, i, hp)
                pend.append((e_sb, d_sb, i, hp))
                if len(pend) > LAG - 1:
                    attn_stage2(v_sb, ot_sb, *pend.pop(0))
            return ot_sb, pend

        def emit_proj(pair, ot_sb, last=False):
            y_sb = yp.tile([128, 4, TP], f32, name="y_sb")
            for ct in range(4):
                p_ps = ps.tile([128, TP], f32, tag="ps1", bufs=3, name="p_ps", padded_shape=(..., 512))
                for jc in range(8):
                    nc.tensor.matmul(
                        p_ps,
                        lhsT=pw_sb[:, jc // 2, jc % 2, ct, :],
                        rhs=ot_sb[:, jc, :TP],
                        start=(jc == 0),
                        stop=(jc == 7),
                    )
                nc.scalar.activation(
                    out=y_sb[:, ct, :],
                    in_=p_ps,
                    func=mybir.ActivationFunctionType.Identity,
                    bias=pb_sb[:, ct : ct + 1],
                )
                if last:
                    # drain the tail: ship each ct slice as soon as it lands
                    nc.sync.dma_start(
                        out=yT[:, pair, ct : ct + 1], in_=y_sb[:, ct : ct + 1]
                    )
            if not last:
                nc.sync.dma_start(out=yT[:, pair], in_=y_sb)

        qkv = {
            0: emit_qkv(0, xp=xp0_sb, fill_after_qk=42),
            1: emit_qkv(1, xp=xp1_sb),
        }
        for pair in range(npair):
            qk_sb, v_sb = qkv.pop(pair)
            ot_sb, pend = emit_attn(qk_sb, v_sb)

            def backlog(pend=pend, v_sb=v_sb, ot_sb=ot_sb):
                for p in pend:
                    attn_stage2(v_sb, ot_sb, *p)

            if pair + 2 < npair:
                qkv[pair + 2] = emit_qkv(pair + 2, mid=backlog)
            else:
                backlog()
                # no qkv to overlap at the tail: pad so the last projections
                # don't idle the PE (and drop its p-state) while the final
                # d-chain completes
                emit_fill(8)
            emit_proj(pair, ot_sb, last=(pair == npair - 1))

    nc.compile()
    return nc


def _prep_weights(qkv_w, qkv_b, proj_w, proj_b, attention_biases, bias_idxs):
    perm_qk = np.empty(512, dtype=np.int64)
    for jp in range(512):
        jt, r = divmod(jp, 128)
        g, d = divmod(r, 32)
        if jt < 2:
            perm_qk[jp] = (jt * 4 + g) * 192 + d
        else:
            perm_qk[jp] = ((jt - 2) * 4 + g) * 192 + 32 + d
    perm_v = np.empty(DH, dtype=np.int64)
    for jv in range(DH):
        h, dv = divmod(jv, 32 * 4)
        perm_v[jv] = h * 192 + 64 + dv

    def hi_lo(w):  # w: [rows, 512] scaled; -> hi, lo, hi/32 as fp8 f32-arrays
        # slice order (hi, lo, hi32): the kernel DMAs slices 0-1 and derives
        # slice 2 = slice0 * 2^-5 on-chip, so hi32 must be the LAST slice
        ws = w * WSCALE
        hi = ws.astype(np.float32).astype(F8)
        hif = hi.astype(np.float32)
        hi32 = (hif / 32.0).astype(F8)
        lo = (ws - hif).astype(np.float32).astype(F8)
        return hi, lo, hi32

    wq = qkv_w[perm_qk]  # [512 j', 512 c]
    parts = hi_lo(wq)
    wqk_ = np.zeros((128, 3, 4, 4, 128), dtype=F8)
    for s, p in enumerate(parts):
        # p [j', c] -> [pp, kc, jt, m]
        wqk_[:, s] = np.ascontiguousarray(
            p.reshape(4, 128, 4, 128).transpose(3, 2, 0, 1)
        )
    qkb_ = np.ascontiguousarray(qkv_b[perm_qk].reshape(4, 128)).astype(np.float32)

    wvm = qkv_w[perm_v]  # [1024 j, 512 c]
    parts = hi_lo(wvm)
    wv_ = np.zeros((128, 3, 4, DH), dtype=F8)
    for s, p in enumerate(parts):
        wv_[:, s] = np.ascontiguousarray(p.reshape(DH, 4, 128).transpose(2, 1, 0))

    pw_ = np.ascontiguousarray(
        proj_w.reshape(4, 128, 8, 128).transpose(3, 2, 0, 1).reshape(128, 4, 2, 4, 128)
    ).astype(BF16)
    bv = qkv_b[perm_v]
    pb_eff = proj_b + proj_w @ bv
    pb_ = np.ascontiguousarray(pb_eff.reshape(4, 128)).astype(np.float32)

    bias_full = attention_biases[:, bias_idxs]  # [H, query, key]
    bkq = np.transpose(bias_full, (0, 2, 1)).astype(np.float64)  # [H, key, query]
    bseed = bkq / SCALE
    bs = bseed.reshape(NUM_HEADS, 2, CH, N_TOK).transpose(2, 0, 1, 3)  # [98, H, 2, 196]
    bp_ = np.zeros((CH, 2, NUM_HEADS, TP), dtype=np.float64)
    bp_[:, 0] = bs.reshape(CH, NUM_HEADS, TP)
    bp_ = bp_.astype(F8)
    id_ = np.zeros((CH, 2, 112), dtype=F8)
    id_[:, 0, :CH][np.arange(CH), np.arange(CH)] = 1.0

    return dict(wqk=wqk_, qkb=qkb_, wv=wv_, pw=pw_, pb=pb_, bp=bp_, idT=id_)


def _prep_x_core(x_core):
    npair = x_core.shape[0] // 2
    xt = x_core.reshape(npair, TP, 4, 128).transpose(3, 0, 2, 1)  # [128, np, 4, 392]
    out = np.zeros((128, npair, 2, 4, 400), dtype=F8)
    hi = xt.astype(np.float32).astype(F8)
    out[:, :, 0, :, :TP] = hi
    out[:, :, 1, :, :TP] = ((xt - hi.astype(np.float32)) * 32.0).astype(F8)
    return out


def _unshard_y(y_core, npair=NPAIR):
    y = np.ascontiguousarray(np.transpose(y_core, (1, 3, 2, 0)))
    return y.reshape(npair * 2, N_TOK, C)


def kernel(x, qkv_w, qkv_b, proj_w, proj_b, attention_biases, bias_idxs):
    x = np.asarray(x, dtype=np.float32)
    qkv_w = np.asarray(qkv_w, dtype=np.float32)
    qkv_b = np.asarray(qkv_b, dtype=np.float32)
    proj_w = np.asarray(proj_w, dtype=np.float32)
    proj_b = np.asarray(proj_b, dtype=np.float32)
    attention_biases = np.asarray(attention_biases, dtype=np.float32)
    bias_idxs = np.asarray(bias_idxs)

    if "nc" not in _cache:
        _cache["nc"] = _build_program()
    nc = _cache["nc"]

    wmap = _prep_weights(qkv_w, qkv_b, proj_w, proj_b, attention_biases, bias_idxs)
    in_maps = []
    for core in range(N_CORES):
        m = dict(wmap)
        m["xT"] = _prep_x_core(x[core * BPC : (core + 1) * BPC])
        in_maps.append(m)

    import os

    guard = {}
    try:
        from antenv import axon_hooks  # noqa: F401
    except ImportError:
        if os.environ.get("BASS_TRACE") and not os.environ.get("BASS_NEVER_TRACE"):
            guard["BASS_NEVER_TRACE"] = True
            os.environ["BASS_NEVER_TRACE"] = "1"
    try:
        res = run_bass_kernel_spmd(nc, in_maps, list(range(N_CORES)))
    finally:
        if guard:
            os.environ.pop("BASS_NEVER_TRACE", None)
    _cache["last_res"] = res
    out = np.concatenate(
        [_unshard_y(res.results[i]["yT"]) for i in range(N_CORES)], axis=0
    )
    return out.astype(np.float32)


if __name__ == "__main__":
    print("building program...")
    _build_program(npair=2)
    print("ok")



# revision 55
# speedup vs baseline: 1.0565x; 1.0121x over previous
"""Trainium2 Bass kernel v4 for AttentionWithBias (LeViT-style attention).

Data-parallel over batch across 8 NeuronCores (32 batches/core, 16 pairs).
Cost-model-driven structure (342.7us -> 328.2us vs the v3 baseline):

  - qk/v projections: 3-term hi/lo fp8e4 DoubleRow (x_hi@w_hi + x_hi@w_lo
    + x_lo@w_hi/32) -> 0.75x bf16 PE cost at ~fp32 accuracy. The w_hi/32
    slice is derived on-chip (exact) to shorten the serial start DMA pipe.
  - attention bias seeded into PSUM by fp8 DoubleRow identity matmul.
  - S^T and O^T in bf16 (98/98 key chunks; fp8 fails the error budget here).
  - per-head 1-bank S PSUM tiles (3 bufs) free a bank so the shared
    qkv/proj PSUM ring can triple-buffer -> no evacuation stalls.
  - softmax denominator: fp16 chunk-add on DVE (2x path, exact here) +
    partition_all_reduce on GPSIMD; normalize: DVE reciprocal + multiply.
    (DVE ALU `divide` would fuse these but the executor rejects it.)
  - v-bias/proj-bias folded on host: pb_eff = proj_b + proj_w @ bv.
  - software-pipelined emission: stage2 lags stage1 by LAG blocks; the
    stage2 backlog is spliced between the next pair's qk and v sections;
    qkv is prefetched two pairs deep at the start.
  - all DMAs on the SP queue in exact need-order (the cost model runs one
    serial DMA pipe); PE warm-up and fill matmuls bridge DMA waits so the
    p-state never drops; the last pair ships y per-ct to shorten the drain.
"""

import sys

sys.path.insert(0, "/opt/trn_rl_repo")

from contextlib import ExitStack

import numpy as np
import ml_dtypes

import concourse.bacc as bacc
import concourse.tile as tile
import concourse.mybir as mybir
import concourse.bass_isa as bass_isa
from concourse.bass_utils import run_bass_kernel_spmd

BF16 = ml_dtypes.bfloat16
F8 = ml_dtypes.float8_e4m3fn
DR = mybir.MatmulPerfMode.DoubleRow

B, N_TOK, C = 256, 196, 512
NUM_HEADS, KEY_DIM, D_V = 8, 32, 128
DH = D_V * NUM_HEADS  # 1024
SCALE = KEY_DIM ** (-0.5)

N_CORES = 8
BPC = B // N_CORES  # 32
NPAIR = BPC // 2  # 16
TP = 2 * N_TOK  # 392
CH = 98  # key chunk

WSCALE = 64.0
N_WARM = 56
LAG = 5

_cache = {}


def _build_program(npair=NPAIR):
    nc = bacc.Bacc("TRN2", target_bir_lowering=False, debug=False)
    f32 = mybir.dt.float32
    f16 = mybir.dt.float16
    bf16 = mybir.dt.bfloat16
    fp8 = mybir.dt.float8e4

    # ---- DRAM I/O ----
    xT = nc.dram_tensor("xT", [128, npair, 2, 4, 400], fp8, kind="ExternalInput").ap()
    wqk = nc.dram_tensor("wqk", [128, 3, 4, 4, 128], fp8, kind="ExternalInput").ap()
    qkb = nc.dram_tensor("qkb", [4, 128], f32, kind="ExternalInput").ap()
    wv = nc.dram_tensor("wv", [128, 3, 4, DH], fp8, kind="ExternalInput").ap()
    pw = nc.dram_tensor("pw", [128, 4, 2, 4, 128], bf16, kind="ExternalInput").ap()
    pb = nc.dram_tensor("pb", [4, 128], f32, kind="ExternalInput").ap()
    idT = nc.dram_tensor("idT", [CH, 2, 112], fp8, kind="ExternalInput").ap()
    bp = nc.dram_tensor("bp", [CH, 2, NUM_HEADS, TP], fp8, kind="ExternalInput").ap()
    yT = nc.dram_tensor("yT", [128, npair, 4, TP], f32, kind="ExternalOutput").ap()

    with tile.TileContext(nc) as tc, ExitStack() as ctx:
        consts = ctx.enter_context(tc.tile_pool(name="consts", bufs=1))
        xio = ctx.enter_context(tc.tile_pool(name="xio", bufs=3))
        qkp = ctx.enter_context(tc.tile_pool(name="qkp", bufs=3))
        vp = ctx.enter_context(tc.tile_pool(name="vp", bufs=3))
        ep = ctx.enter_context(tc.tile_pool(name="ep", bufs=7))
        dp = ctx.enter_context(tc.tile_pool(name="dp", bufs=11))
        rp = ctx.enter_context(tc.tile_pool(name="rp", bufs=5))
        op = ctx.enter_context(tc.tile_pool(name="op", bufs=2))
        yp = ctx.enter_context(tc.tile_pool(name="yp", bufs=2))
        ps = ctx.enter_context(tc.tile_pool(name="ps", bufs=1, space="PSUM"))

        # ---- constants ----
        # The cost model runs all DMA transfers through ONE serial pipe, in
        # HWDGE arrival order. Everything goes on the SP queue in exactly the
        # order the PE will need it: wqk -> x(pair0) -> qkb -> wv(hi,lo) ->
        # wv(hi32) -> x(pair1) -> bias tables -> proj weights.
        wqk_sb = consts.tile([128, 3, 4, 4, 128], fp8)
        nc.sync.dma_start(out=wqk_sb[:, :2], in_=wqk[:, :2])
        xp0_sb = xio.tile([128, 2, 4, 400], fp8, name="xp")
        nc.sync.dma_start(out=xp0_sb, in_=xT[:, 0])
        qkb_sb = consts.tile([128, 4], f32)
        nc.sync.dma_start(out=qkb_sb, in_=qkb.rearrange("k p -> p k"))
        wv_sb = consts.tile([128, 3, 4, DH], fp8)
        # f-halves: the first half of DH lands ~1.6us earlier on the serial
        # DMA pipe, so the (f-major-ordered) v matmuls can start sooner
        nc.sync.dma_start(out=wv_sb[:, :2, :, :512], in_=wv[:, :2, :, :512])
        nc.sync.dma_start(out=wv_sb[:, :2, :, 512:], in_=wv[:, :2, :, 512:])
        xp1_sb = xio.tile([128, 2, 4, 400], fp8, name="xp")
        nc.sync.dma_start(out=xp1_sb, in_=xT[:, 1])
        # hi32 = hi * 2^-5 is derived on-chip (exact: fp8 -> f32 -> scale ->
        # fp8 round matches the host computation), keeping the serial DMA
        # pipe ~3.5us shorter at the critical kernel start. The wv derives
        # are emitted inside emit_qkv(0), after the pair-0 qk evacuations,
        # to avoid head-of-line blocking DVE's queue.
        nc.vector.tensor_scalar_mul(
            out=wqk_sb[:, 2], in0=wqk_sb[:, 0], scalar1=1.0 / 32.0
        )
        bp_sb = consts.tile([CH, 2, NUM_HEADS, TP], fp8)
        nc.sync.dma_start(out=bp_sb, in_=bp)
        id_sb = consts.tile([CH, 2, 112], fp8)
        nc.sync.dma_start(out=id_sb, in_=idT)
        pw_sb = consts.tile([128, 4, 2, 4, 128], bf16)
        nc.sync.dma_start(out=pw_sb, in_=pw)
        pb_sb = consts.tile([128, 4], f32)
        nc.sync.dma_start(out=pb_sb, in_=pb.rearrange("k p -> p k"))
        wm_sb = consts.tile([128, 128], bf16)
        nc.vector.memset(wm_sb, 1.0)

        # PE warm-up: p-state ramps to full rate after ~3us of continuous busy.
        def emit_fill(n):
            warm_ps = ps.tile([128, 512], f32, tag="ps1", bufs=3, name="warm_ps")
            for w in range(n):
                nc.tensor.matmul(
                    warm_ps[:, :128], lhsT=wm_sb, rhs=wm_sb, start=True, stop=True
                )

        emit_fill(N_WARM)

        def emit_qkv(pair, mid=None, xp=None, fill_after_qk=0):
            if xp is None:
                xp = xio.tile([128, 2, 4, 400], fp8, name="xp")
                nc.sync.dma_start(out=xp, in_=xT[:, pair])

            # ---- qk projection: 3-term fp8 DR ----
            qk_sb = qkp.tile([128, 5, 400], bf16, name="qk_sb")
            # (x hi/lo, w hi/lo/hi32); the hi32 term last so the derived wv
            # hi32 slice isn't needed until each gen's end
            terms = [(0, 0), (0, 1), (1, 2)]
            for jt in range(4):
                qk_ps = ps.tile([128, TP], f32, tag="ps1", bufs=3, name="qk_ps", padded_shape=(..., 512))
                n = 0
                for xs, wsd in terms:
                    for t in range(2):
                        nc.tensor.matmul(
                            qk_ps,
                            lhsT=wqk_sb[:, wsd, 2 * t : 2 * t + 2, jt, :],
                            rhs=xp[:, xs, 2 * t : 2 * t + 2, :TP],
                            start=(n == 0),
                            stop=(n == 5),
                            perf_mode=DR,
                        )
                        n += 1
                nc.vector.tensor_scalar(
                    out=qk_sb[:, jt, :TP],
                    in0=qk_ps,
                    scalar1=1.0 / WSCALE,
                    scalar2=qkb_sb[:, jt : jt + 1],
                    op0=mybir.AluOpType.mult,
                    op1=mybir.AluOpType.add,
                )

            if mid is not None:
                # the previous pair's stage2 backlog slots in here, so its
                # DVE norms land ahead of the v evacuations in DVE's queue
                mid()
            if pair == 0:
                nc.vector.tensor_scalar_mul(
                    out=wv_sb[:, 2, :, :512], in0=wv_sb[:, 0, :, :512], scalar1=1.0 / 32.0
                )
                # second half on Act: it is idle here, and on DVE this would
                # head-of-line block the first v evacuations behind the
                # still-in-flight wv second-half DMA
                nc.scalar.activation(
                    out=wv_sb[:, 2, :, 512:],
                    in_=wv_sb[:, 0, :, 512:],
                    func=mybir.ActivationFunctionType.Copy,
                    scale=1.0 / 32.0,
                )
            if fill_after_qk:
                # pair 0 only: the wv DMA is still in flight when the qk
                # matmuls finish; keep the PE p-state warm until it lands
                emit_fill(fill_after_qk)

            # ---- v projection: 3-term fp8 DR ----
            # pair 0 runs f-major (all DH-first-half gens first) to match the
            # wv DMA/derive split landing order; later pairs keep cc-major,
            # which schedules better against the stage2 backlog
            v_sb = vp.tile([CH, 4, DH], bf16, name="v_sb")
            gens = (
                [(f, cc) for f in range(2) for cc in range(4)]
                if pair == 0
                else [(f, cc) for cc in range(4) for f in range(2)]
            )
            for f, cc in gens:
                    tok0 = CH * cc
                    v_ps = ps.tile([CH, 512], f32, tag="ps1", bufs=3, name="v_ps")
                    n = 0
                    for xs, wsd in terms:
                        for t in range(2):
                            nc.tensor.matmul(
                                v_ps,
                                lhsT=xp[:, xs, 2 * t : 2 * t + 2, tok0 : tok0 + CH],
                                rhs=wv_sb[:, wsd, 2 * t : 2 * t + 2, 512 * f : 512 * (f + 1)],
                                start=(n == 0),
                                stop=(n == 5),
                                perf_mode=DR,
                            )
                            n += 1
                    vdst = v_sb[:, cc, 512 * f : 512 * (f + 1)]
                    nc.vector.tensor_scalar_mul(out=vdst, in0=v_ps, scalar1=1.0 / WSCALE)
            return qk_sb, v_sb

        eb_count = [0]

        def attn_stage1(qk_sb, i, hp):
            t0 = N_TOK * i
            e_sb = ep.tile([128, 2, 2, 208], bf16, name="e_sb")
            if eb_count[0] < 7:
                # zero partitions 96..127 on every ring slot (ep bufs=7) so the
                # 128-channel partition reduction reads zeros there; later
                # generations reuse the same bytes untouched. Pool is idle in
                # the ramp-up phase, so these don't block the evac engines.
                nc.gpsimd.memset(e_sb[96:, :, :, :], 0.0)
                eb_count[0] += 1
            for hh in range(2):
                h = 2 * hp + hh
                # per-head 1-bank PSUM tile (vs 2-bank for the head pair):
                # frees a PSUM bank so the shared ps1 ring can triple-buffer
                s_ps = ps.tile([CH, 2, N_TOK], f32, tag="psS", bufs=3, name="s_ps", padded_shape=(..., 208))
                nc.tensor.matmul(
                    s_ps,
                    lhsT=id_sb[:, :, :CH],
                    rhs=bp_sb[:, :, h, :],
                    start=True,
                    stop=False,
                    perf_mode=DR,
                )
                g = h % 4
                jq = h // 4
                jk = 2 + h // 4
                p0 = 32 * g
                q_rhs = qk_sb[p0 : p0 + 32, jq, t0 : t0 + N_TOK]
                for cc in range(2):
                    k_lhs = qk_sb[p0 : p0 + 32, jk, t0 + CH * cc : t0 + CH * (cc + 1)]
                    nc.tensor.matmul(
                        s_ps[:, cc, :],
                        lhsT=k_lhs,
                        rhs=q_rhs,
                        start=False,
                        stop=True,
                        skip_group_check=True,
                        tile_position=(p0, 0),
                    )
                nc.scalar.activation(
                    out=e_sb[:CH, hh, :, :N_TOK],
                    in_=s_ps,
                    func=mybir.ActivationFunctionType.Exp,
                    scale=SCALE,
                )
            # denominator: chunk-add on DVE (all-2-byte operands hit the 2x
            # path; fp16 keeps ~11 mantissa bits and cannot overflow here),
            # then partition all-reduce on Pool (f32 accumulate)
            ee_sb = dp.tile([128, 2, N_TOK], f16, name="ee_sb")
            nc.vector.tensor_add(
                out=ee_sb, in0=e_sb[:, :, 0, :N_TOK], in1=e_sb[:, :, 1, :N_TOK]
            )
            d_sb = dp.tile([128, 2, N_TOK], f32, name="d_sb")
            nc.gpsimd.partition_all_reduce(
                out_ap=d_sb, in_ap=ee_sb, channels=128, reduce_op=bass_isa.ReduceOp.add
            )
            return e_sb, d_sb

        def attn_stage2(v_sb, ot_sb, e_sb, d_sb, i, hp):
            t0 = N_TOK * i
            o_ps = ps.tile([128, 2, N_TOK], f32, tag="psOD", bufs=2, name="o_ps", padded_shape=(..., 256))
            for hh in range(2):
                h = 2 * hp + hh
                for cc in range(2):
                    nc.tensor.matmul(
                        o_ps[:, hh, :],
                        lhsT=v_sb[:, 2 * i + cc, 128 * h : 128 * (h + 1)],
                        rhs=e_sb[:CH, hh, cc, :N_TOK],
                        start=(cc == 0),
                        stop=(cc == 1),
                    )
            rec_sb = rp.tile([128, 2, N_TOK], f32, name="rec_sb")
            nc.vector.reciprocal_approx_fast(out=rec_sb, in_=d_sb)
            nc.vector.tensor_mul(
                out=ot_sb[:, 2 * hp : 2 * hp + 2, t0 : t0 + N_TOK],
                in0=o_ps,
                in1=rec_sb,
            )

        def emit_attn(qk_sb, v_sb, ot_sb=None, pend=None, start_block=0, end_block=8):
            if ot_sb is None:
                ot_sb = op.tile([128, NUM_HEADS, 400], bf16, name="ot_sb")
            if pend is None:
                pend = []
            blocks = [(i, hp) for i in range(2) for hp in range(NUM_HEADS // 2)]
            for i, hp in blocks[start_block:end_block]:
                e_sb, d_sb = attn_stage1(qk_sb, i, hp)
                pend.append((e_sb, d_sb, i, hp))
                if len(pend) > LAG - 1:
                    attn_stage2(v_sb, ot_sb, *pend.pop(0))
            return ot_sb, pend

        def emit_proj(pair, ot_sb, last=False):
            y_sb = yp.tile([128, 4, TP], f32, name="y_sb")
            for ct in range(4):
                p_ps = ps.tile([128, TP], f32, tag="ps1", bufs=3, name="p_ps", padded_shape=(..., 512))
                for jc in range(8):
                    nc.tensor.matmul(
                        p_ps,
                        lhsT=pw_sb[:, jc // 2, jc % 2, ct, :],
                        rhs=ot_sb[:, jc, :TP],
                        start=(jc == 0),
                        stop=(jc == 7),
                    )
                nc.scalar.activation(
                    out=y_sb[:, ct, :],
                    in_=p_ps,
                    func=mybir.ActivationFunctionType.Identity,
                    bias=pb_sb[:, ct : ct + 1],
                )
                if last:
                    # drain the tail: ship each ct slice as soon as it lands
                    nc.sync.dma_start(
                        out=yT[:, pair, ct : ct + 1], in_=y_sb[:, ct : ct + 1]
                    )
            if not last:
                nc.sync.dma_start(out=yT[:, pair], in_=y_sb)

        qkv = {
            0: emit_qkv(0, xp=xp0_sb, fill_after_qk=15),
            1: emit_qkv(1, xp=xp1_sb),
        }
        carry = {}
        for pair in range(npair):
            qk_sb, v_sb = qkv.pop(pair)
            if pair in carry:
                c_ot, c_pend = carry.pop(pair)
                ot_sb, pend = emit_attn(
                    qk_sb, v_sb, ot_sb=c_ot, pend=c_pend, start_block=8
                )
            else:
                ot_sb, pend = emit_attn(qk_sb, v_sb)

            def backlog(pend=pend, v_sb=v_sb, ot_sb=ot_sb):
                for p in pend:
                    attn_stage2(v_sb, ot_sb, *p)

            if pair + 2 < npair:
                qkv[pair + 2] = emit_qkv(pair + 2, mid=backlog)
            else:
                backlog()
                if pair + 1 < npair:
                    # the tail is exp-throughput-bound on Act: pre-emit the
                    # last pair's first stage1 blocks so their exps overlap
                    # this pair's projection instead of serializing after it
                    nqk, nv = qkv[pair + 1]
                    carry[pair + 1] = emit_attn(nqk, nv, end_block=8)
                # no qkv to overlap at the tail: pad so the last projections
                # don't idle the PE (and drop its p-state) while the final
                # d-chain completes
                emit_fill(8)
            emit_proj(pair, ot_sb, last=(pair == npair - 1))

    nc.compile()
    return nc


def _prep_weights(qkv_w, qkv_b, proj_w, proj_b, attention_biases, bias_idxs):
    perm_qk = np.empty(512, dtype=np.int64)
    for jp in range(512):
        jt, r = divmod(jp, 128)
        g, d = divmod(r, 32)
        if jt < 2:
            perm_qk[jp] = (jt * 4 + g) * 192 + d
        else:
            perm_qk[jp] = ((jt - 2) * 4 + g) * 192 + 32 + d
    perm_v = np.empty(DH, dtype=np.int64)
    for jv in range(DH):
        h, dv = divmod(jv, 32 * 4)
        perm_v[jv] = h * 192 + 64 + dv

    def hi_lo(w):  # w: [rows, 512] scaled; -> hi, lo, hi/32 as fp8 f32-arrays
        # slice order (hi, lo, hi32): the kernel DMAs slices 0-1 and derives
        # slice 2 = slice0 * 2^-5 on-chip, so hi32 must be the LAST slice
        ws = w * WSCALE
        hi = ws.astype(np.float32).astype(F8)
        hif = hi.astype(np.float32)
        hi32 = (hif / 32.0).astype(F8)
        lo = (ws - hif).astype(np.float32).astype(F8)
        return hi, lo, hi32

    wq = qkv_w[perm_qk]  # [512 j', 512 c]
    parts = hi_lo(wq)
    wqk_ = np.zeros((128, 3, 4, 4, 128), dtype=F8)
    for s, p in enumerate(parts):
        # p [j', c] -> [pp, kc, jt, m]
        wqk_[:, s] = np.ascontiguousarray(
            p.reshape(4, 128, 4, 128).transpose(3, 2, 0, 1)
        )
    qkb_ = np.ascontiguousarray(qkv_b[perm_qk].reshape(4, 128)).astype(np.float32)

    wvm = qkv_w[perm_v]  # [1024 j, 512 c]
    parts = hi_lo(wvm)
    wv_ = np.zeros((128, 3, 4, DH), dtype=F8)
    for s, p in enumerate(parts):
        wv_[:, s] = np.ascontiguousarray(p.reshape(DH, 4, 128).transpose(2, 1, 0))

    pw_ = np.ascontiguousarray(
        proj_w.reshape(4, 128, 8, 128).transpose(3, 2, 0, 1).reshape(128, 4, 2, 4, 128)
    ).astype(BF16)
    bv = qkv_b[perm_v]
    pb_eff = proj_b + proj_w @ bv
    pb_ = np.ascontiguousarray(pb_eff.reshape(4, 128)).astype(np.float32)

    bias_full = attention_biases[:, bias_idxs]  # [H, query, key]
    bkq = np.transpose(bias_full, (0, 2, 1)).astype(np.float64)  # [H, key, query]
    bseed = bkq / SCALE
    bs = bseed.reshape(NUM_HEADS, 2, CH, N_TOK).transpose(2, 0, 1, 3)  # [98, H, 2, 196]
    bp_ = np.zeros((CH, 2, NUM_HEADS, TP), dtype=np.float64)
    bp_[:, 0] = bs.reshape(CH, NUM_HEADS, TP)
    bp_ = bp_.astype(F8)
    id_ = np.zeros((CH, 2, 112), dtype=F8)
    id_[:, 0, :CH][np.arange(CH), np.arange(CH)] = 1.0

    return dict(wqk=wqk_, qkb=qkb_, wv=wv_, pw=pw_, pb=pb_, bp=bp_, idT=id_)


def _prep_x_core(x_core):
    npair = x_core.shape[0] // 2
    xt = x_core.reshape(npair, TP, 4, 128).transpose(3, 0, 2, 1)  # [128, np, 4, 392]
    out = np.zeros((128, npair, 2, 4, 400), dtype=F8)
    hi = xt.astype(np.float32).astype(F8)
    out[:, :, 0, :, :TP] = hi
    out[:, :, 1, :, :TP] = ((xt - hi.astype(np.float32)) * 32.0).astype(F8)
    return out


def _unshard_y(y_core, npair=NPAIR):
    y = np.ascontiguousarray(np.transpose(y_core, (1, 3, 2, 0)))
    return y.reshape(npair * 2, N_TOK, C)


def kernel(x, qkv_w, qkv_b, proj_w, proj_b, attention_biases, bias_idxs):
    x = np.asarray(x, dtype=np.float32)
    qkv_w = np.asarray(qkv_w, dtype=np.float32)
    qkv_b = np.asarray(qkv_b, dtype=np.float32)
    proj_w = np.asarray(proj_w, dtype=np.float32)
    proj_b = np.asarray(proj_b, dtype=np.float32)
    attention_biases = np.asarray(attention_biases, dtype=np.float32)
    bias_idxs = np.asarray(bias_idxs)

    if "nc" not in _cache:
        _cache["nc"] = _build_program()
    nc = _cache["nc"]

    wmap = _prep_weights(qkv_w, qkv_b, proj_w, proj_b, attention_biases, bias_idxs)
    in_maps = []
    for core in range(N_CORES):
        m = dict(wmap)
        m["xT"] = _prep_x_core(x[core * BPC : (core + 1) * BPC])
        in_maps.append(m)

    import os

    guard = {}
    try:
        from antenv import axon_hooks  # noqa: F401
    except ImportError:
        if os.environ.get("BASS_TRACE") and not os.environ.get("BASS_NEVER_TRACE"):
            guard["BASS_NEVER_TRACE"] = True
            os.environ["BASS_NEVER_TRACE"] = "1"
    try:
        res = run_bass_kernel_spmd(nc, in_maps, list(range(N_CORES)))
    finally:
        if guard:
            os.environ.pop("BASS_NEVER_TRACE", None)
    _cache["last_res"] = res
    out = np.concatenate(
        [_unshard_y(res.results[i]["yT"]) for i in range(N_CORES)], axis=0
    )
    return out.astype(np.float32)


if __name__ == "__main__":
    print("building program...")
    _build_program(npair=2)
    print("ok")



# revision 72
# speedup vs baseline: 1.0637x; 1.0069x over previous
"""Trainium2 Bass kernel v4 for AttentionWithBias (LeViT-style attention).

Data-parallel over batch across 8 NeuronCores (32 batches/core, 16 pairs).
Cost-model-driven structure (342.7us -> 322.1us vs the v3 baseline):

  - qk/v projections: 3-term hi/lo fp8e4 DoubleRow (x_hi@w_hi + x_hi@w_lo
    + x_lo@w_hi/32) -> 0.75x bf16 PE cost at ~fp32 accuracy. wqk's w_hi/32
    slice is derived on-chip (exact) to shorten the serial start DMA pipe;
    wv arrives in DH-halves (hi/lo then hi32 per half) in exact need-order.
  - attention bias seeded into PSUM by fp8 DoubleRow identity matmul.
  - S^T and O^T in bf16 (98/98 key chunks; fp8 fails the error budget here).
  - per-head 1-bank S PSUM tiles (3 bufs) free a bank so the shared
    qkv/proj PSUM ring can triple-buffer -> no evacuation stalls.
  - softmax denominator: fp16 chunk-add on DVE (2x path, exact here) +
    partition_all_reduce on GPSIMD; normalize: DVE reciprocal + multiply.
    (DVE ALU `divide` would fuse these but the executor rejects it.)
  - v-bias/proj-bias folded on host: pb_eff = proj_b + proj_w @ bv.
  - software-pipelined emission: stage2 lags stage1 by LAG blocks; the
    stage2 backlog is spliced between the next pair's qk and v sections;
    qkv is prefetched two pairs deep at the start.
  - all DMAs on the SP queue in exact need-order (the cost model runs one
    serial DMA pipe); PE warm-up and fill matmuls bridge DMA waits so the
    p-state never drops; the last pair ships y per-ct to shorten the drain.
  - the last pair's whole attention is pre-emitted during the previous
    pair's projection window: the tail is exp-throughput-bound on Act, so
    its 16 exps start ~5us earlier and the final projection never stalls.
"""

import sys

sys.path.insert(0, "/opt/trn_rl_repo")

from contextlib import ExitStack

import numpy as np
import ml_dtypes

import concourse.bacc as bacc
import concourse.tile as tile
import concourse.mybir as mybir
import concourse.bass_isa as bass_isa
from concourse.bass_utils import run_bass_kernel_spmd

BF16 = ml_dtypes.bfloat16
F8 = ml_dtypes.float8_e4m3fn
DR = mybir.MatmulPerfMode.DoubleRow

B, N_TOK, C = 256, 196, 512
NUM_HEADS, KEY_DIM, D_V = 8, 32, 128
DH = D_V * NUM_HEADS  # 1024
SCALE = KEY_DIM ** (-0.5)

N_CORES = 8
BPC = B // N_CORES  # 32
NPAIR = BPC // 2  # 16
TP = 2 * N_TOK  # 392
CH = 98  # key chunk

WSCALE = 64.0
N_WARM = 42
LAG = 5

_cache = {}


def _build_program(npair=NPAIR):
    nc = bacc.Bacc("TRN2", target_bir_lowering=False, debug=False)
    f32 = mybir.dt.float32
    f16 = mybir.dt.float16
    bf16 = mybir.dt.bfloat16
    fp8 = mybir.dt.float8e4

    # ---- DRAM I/O ----
    xT = nc.dram_tensor("xT", [128, npair, 2, 4, 400], fp8, kind="ExternalInput").ap()
    wqk = nc.dram_tensor("wqk", [128, 3, 4, 4, 128], fp8, kind="ExternalInput").ap()
    qkb = nc.dram_tensor("qkb", [4, 128], f32, kind="ExternalInput").ap()
    wv = nc.dram_tensor("wv", [128, 3, 4, DH], fp8, kind="ExternalInput").ap()
    pw = nc.dram_tensor("pw", [128, 4, 2, 4, 128], bf16, kind="ExternalInput").ap()
    pb = nc.dram_tensor("pb", [4, 128], f32, kind="ExternalInput").ap()
    idT = nc.dram_tensor("idT", [CH, 2, 112], fp8, kind="ExternalInput").ap()
    bp = nc.dram_tensor("bp", [CH, 2, NUM_HEADS, TP], fp8, kind="ExternalInput").ap()
    yT = nc.dram_tensor("yT", [128, npair, 4, TP], f32, kind="ExternalOutput").ap()

    with tile.TileContext(nc) as tc, ExitStack() as ctx:
        consts = ctx.enter_context(tc.tile_pool(name="consts", bufs=1))
        xio = ctx.enter_context(tc.tile_pool(name="xio", bufs=3))
        qkp = ctx.enter_context(tc.tile_pool(name="qkp", bufs=3))
        vp = ctx.enter_context(tc.tile_pool(name="vp", bufs=3))
        ep = ctx.enter_context(tc.tile_pool(name="ep", bufs=7))
        dp = ctx.enter_context(tc.tile_pool(name="dp", bufs=11))
        rp = ctx.enter_context(tc.tile_pool(name="rp", bufs=5))
        op = ctx.enter_context(tc.tile_pool(name="op", bufs=2))
        yp = ctx.enter_context(tc.tile_pool(name="yp", bufs=2))
        ps = ctx.enter_context(tc.tile_pool(name="ps", bufs=1, space="PSUM"))

        # ---- constants ----
        # The cost model runs all DMA transfers through ONE serial pipe, in
        # HWDGE arrival order. Everything goes on the SP queue in exactly the
        # order the PE will need it: wqk -> x(pair0) -> qkb -> wv(hi,lo) ->
        # wv(hi32) -> x(pair1) -> bias tables -> proj weights.
        wqk_sb = consts.tile([128, 3, 4, 4, 128], fp8)
        nc.sync.dma_start(out=wqk_sb[:, :2], in_=wqk[:, :2])
        xp0_sb = xio.tile([128, 2, 4, 400], fp8, name="xp")
        nc.sync.dma_start(out=xp0_sb, in_=xT[:, 0])
        qkb_sb = consts.tile([128, 4], f32)
        nc.sync.dma_start(out=qkb_sb, in_=qkb.rearrange("k p -> p k"))
        wv_sb = consts.tile([128, 3, 4, DH], fp8)
        # f-halves in need-order: each half's hi/lo slices then its hi32
        # slice, so the (f-major-ordered) pair-0 v matmuls start as soon as
        # the first half lands on the serial DMA pipe
        nc.sync.dma_start(out=wv_sb[:, :2, :, :512], in_=wv[:, :2, :, :512])
        nc.sync.dma_start(out=wv_sb[:, 2, :, :512], in_=wv[:, 2, :, :512])
        nc.sync.dma_start(out=wv_sb[:, :2, :, 512:], in_=wv[:, :2, :, 512:])
        nc.sync.dma_start(out=wv_sb[:, 2, :, 512:], in_=wv[:, 2, :, 512:])
        # wqk's hi32 = hi * 2^-5 is derived on-chip instead (exact: fp8 ->
        # f32 -> scale -> fp8 round matches the host computation), keeping
        # the pipe shorter before x arrives
        nc.vector.tensor_scalar_mul(
            out=wqk_sb[:, 2], in0=wqk_sb[:, 0], scalar1=1.0 / 32.0
        )
        bp_sb = consts.tile([CH, 2, NUM_HEADS, TP], fp8)
        nc.sync.dma_start(out=bp_sb, in_=bp)
        id_sb = consts.tile([CH, 2, 112], fp8)
        nc.sync.dma_start(out=id_sb, in_=idT)
        xp1_sb = xio.tile([128, 2, 4, 400], fp8, name="xp")
        nc.sync.dma_start(out=xp1_sb, in_=xT[:, 1])
        pw_sb = consts.tile([128, 4, 2, 4, 128], bf16)
        nc.sync.dma_start(out=pw_sb, in_=pw)
        pb_sb = consts.tile([128, 4], f32)
        nc.sync.dma_start(out=pb_sb, in_=pb.rearrange("k p -> p k"))
        wm_sb = consts.tile([128, 128], bf16)
        nc.vector.memset(wm_sb, 1.0)

        # PE warm-up: p-state ramps to full rate after ~3us of continuous busy.
        def emit_fill(n):
            warm_ps = ps.tile([128, 512], f32, tag="ps1", bufs=3, name="warm_ps")
            for w in range(n):
                nc.tensor.matmul(
                    warm_ps[:, :128], lhsT=wm_sb, rhs=wm_sb, start=True, stop=True
                )

        emit_fill(N_WARM)

        def emit_qkv(pair, mid=None, xp=None, fill_after_qk=0):
            if xp is None:
                xp = xio.tile([128, 2, 4, 400], fp8, name="xp")
                nc.sync.dma_start(out=xp, in_=xT[:, pair])

            # ---- qk projection: 3-term fp8 DR ----
            qk_sb = qkp.tile([128, 5, 400], bf16, name="qk_sb")
            # (x hi/lo, w hi/lo/hi32); the hi32 term last so the derived wv
            # hi32 slice isn't needed until each gen's end
            terms = [(0, 0), (0, 1), (1, 2)]
            for jt in range(4):
                qk_ps = ps.tile([128, TP], f32, tag="ps1", bufs=3, name="qk_ps", padded_shape=(..., 512))
                n = 0
                for xs, wsd in terms:
                    for t in range(2):
                        nc.tensor.matmul(
                            qk_ps,
                            lhsT=wqk_sb[:, wsd, 2 * t : 2 * t + 2, jt, :],
                            rhs=xp[:, xs, 2 * t : 2 * t + 2, :TP],
                            start=(n == 0),
                            stop=(n == 5),
                            perf_mode=DR,
                        )
                        n += 1
                nc.vector.tensor_scalar(
                    out=qk_sb[:, jt, :TP],
                    in0=qk_ps,
                    scalar1=1.0 / WSCALE,
                    scalar2=qkb_sb[:, jt : jt + 1],
                    op0=mybir.AluOpType.mult,
                    op1=mybir.AluOpType.add,
                )

            if mid is not None:
                # the previous pair's stage2 backlog slots in here, so its
                # DVE norms land ahead of the v evacuations in DVE's queue
                mid()
            if fill_after_qk:
                # pair 0 only: the wv DMA is still in flight when the qk
                # matmuls finish; keep the PE p-state warm until it lands
                emit_fill(fill_after_qk)

            # ---- v projection: 3-term fp8 DR ----
            # pair 0 runs f-major (all DH-first-half gens first) to match the
            # wv DMA/derive split landing order; later pairs keep cc-major,
            # which schedules better against the stage2 backlog
            v_sb = vp.tile([CH, 4, DH], bf16, name="v_sb")
            gens = (
                [(f, cc) for f in range(2) for cc in range(4)]
                if pair == 0
                else [(f, cc) for cc in range(4) for f in range(2)]
            )
            for f, cc in gens:
                    tok0 = CH * cc
                    v_ps = ps.tile([CH, 512], f32, tag="ps1", bufs=3, name="v_ps")
                    n = 0
                    for xs, wsd in terms:
                        for t in range(2):
                            nc.tensor.matmul(
                                v_ps,
                                lhsT=xp[:, xs, 2 * t : 2 * t + 2, tok0 : tok0 + CH],
                                rhs=wv_sb[:, wsd, 2 * t : 2 * t + 2, 512 * f : 512 * (f + 1)],
                                start=(n == 0),
                                stop=(n == 5),
                                perf_mode=DR,
                            )
                            n += 1
                    vdst = v_sb[:, cc, 512 * f : 512 * (f + 1)]
                    nc.vector.tensor_scalar_mul(out=vdst, in0=v_ps, scalar1=1.0 / WSCALE)
            return qk_sb, v_sb

        eb_count = [0]

        def attn_stage1(qk_sb, i, hp):
            t0 = N_TOK * i
            e_sb = ep.tile([128, 2, 2, 208], bf16, name="e_sb")
            if eb_count[0] < 7:
                # zero partitions 96..127 on every ring slot (ep bufs=7) so the
                # 128-channel partition reduction reads zeros there; later
                # generations reuse the same bytes untouched. Pool is idle in
                # the ramp-up phase, so these don't block the evac engines.
                nc.gpsimd.memset(e_sb[96:, :, :, :], 0.0)
                eb_count[0] += 1
            for hh in range(2):
                h = 2 * hp + hh
                # per-head 1-bank PSUM tile (vs 2-bank for the head pair):
                # frees a PSUM bank so the shared ps1 ring can triple-buffer
                s_ps = ps.tile([CH, 2, N_TOK], f32, tag="psS", bufs=3, name="s_ps", padded_shape=(..., 208))
                nc.tensor.matmul(
                    s_ps,
                    lhsT=id_sb[:, :, :CH],
                    rhs=bp_sb[:, :, h, :],
                    start=True,
                    stop=False,
                    perf_mode=DR,
                )
                g = h % 4
                jq = h // 4
                jk = 2 + h // 4
                p0 = 32 * g
                q_rhs = qk_sb[p0 : p0 + 32, jq, t0 : t0 + N_TOK]
                for cc in range(2):
                    k_lhs = qk_sb[p0 : p0 + 32, jk, t0 + CH * cc : t0 + CH * (cc + 1)]
                    nc.tensor.matmul(
                        s_ps[:, cc, :],
                        lhsT=k_lhs,
                        rhs=q_rhs,
                        start=False,
                        stop=True,
                        skip_group_check=True,
                        tile_position=(p0, 0),
                    )
                nc.scalar.activation(
                    out=e_sb[:CH, hh, :, :N_TOK],
                    in_=s_ps,
                    func=mybir.ActivationFunctionType.Exp,
                    scale=SCALE,
                )
            # denominator: chunk-add on DVE (all-2-byte operands hit the 2x
            # path; fp16 keeps ~11 mantissa bits and cannot overflow here),
            # then partition all-reduce on Pool (f32 accumulate)
            ee_sb = dp.tile([128, 2, N_TOK], f16, name="ee_sb")
            nc.vector.tensor_add(
                out=ee_sb, in0=e_sb[:, :, 0, :N_TOK], in1=e_sb[:, :, 1, :N_TOK]
            )
            d_sb = dp.tile([128, 2, N_TOK], f32, name="d_sb")
            nc.gpsimd.partition_all_reduce(
                out_ap=d_sb, in_ap=ee_sb, channels=128, reduce_op=bass_isa.ReduceOp.add
            )
            return e_sb, d_sb

        def attn_stage2(v_sb, ot_sb, e_sb, d_sb, i, hp):
            t0 = N_TOK * i
            o_ps = ps.tile([128, 2, N_TOK], f32, tag="psOD", bufs=2, name="o_ps", padded_shape=(..., 256))
            for hh in range(2):
                h = 2 * hp + hh
                for cc in range(2):
                    nc.tensor.matmul(
                        o_ps[:, hh, :],
                        lhsT=v_sb[:, 2 * i + cc, 128 * h : 128 * (h + 1)],
                        rhs=e_sb[:CH, hh, cc, :N_TOK],
                        start=(cc == 0),
                        stop=(cc == 1),
                    )
            rec_sb = rp.tile([128, 2, N_TOK], f32, name="rec_sb")
            nc.vector.reciprocal_approx_fast(out=rec_sb, in_=d_sb)
            nc.vector.tensor_mul(
                out=ot_sb[:, 2 * hp : 2 * hp + 2, t0 : t0 + N_TOK],
                in0=o_ps,
                in1=rec_sb,
            )

        def emit_attn(qk_sb, v_sb, ot_sb=None, pend=None, start_block=0, end_block=8):
            if ot_sb is None:
                ot_sb = op.tile([128, NUM_HEADS, 400], bf16, name="ot_sb")
            if pend is None:
                pend = []
            blocks = [(i, hp) for i in range(2) for hp in range(NUM_HEADS // 2)]
            for i, hp in blocks[start_block:end_block]:
                e_sb, d_sb = attn_stage1(qk_sb, i, hp)
                pend.append((e_sb, d_sb, i, hp))
                if len(pend) > LAG - 1:
                    attn_stage2(v_sb, ot_sb, *pend.pop(0))
            return ot_sb, pend

        def emit_proj(pair, ot_sb, last=False):
            y_sb = yp.tile([128, 4, TP], f32, name="y_sb")
            for ct in range(4):
                p_ps = ps.tile([128, TP], f32, tag="ps1", bufs=3, name="p_ps", padded_shape=(..., 512))
                for jc in range(8):
                    nc.tensor.matmul(
                        p_ps,
                        lhsT=pw_sb[:, jc // 2, jc % 2, ct, :],
                        rhs=ot_sb[:, jc, :TP],
                        start=(jc == 0),
                        stop=(jc == 7),
                    )
                nc.scalar.activation(
                    out=y_sb[:, ct, :],
                    in_=p_ps,
                    func=mybir.ActivationFunctionType.Identity,
                    bias=pb_sb[:, ct : ct + 1],
                )
                if last:
                    # drain the tail: ship each ct slice as soon as it lands
                    nc.sync.dma_start(
                        out=yT[:, pair, ct : ct + 1], in_=y_sb[:, ct : ct + 1]
                    )
            if not last:
                nc.sync.dma_start(out=yT[:, pair], in_=y_sb)

        qkv = {
            0: emit_qkv(0, xp=xp0_sb, fill_after_qk=0),
            1: emit_qkv(1, xp=xp1_sb),
        }
        carry = {}
        for pair in range(npair):
            qk_sb, v_sb = qkv.pop(pair)
            if pair in carry:
                c_ot, c_pend = carry.pop(pair)
                ot_sb, pend = emit_attn(
                    qk_sb, v_sb, ot_sb=c_ot, pend=c_pend, start_block=8
                )
            else:
                ot_sb, pend = emit_attn(qk_sb, v_sb)

            def backlog(pend=pend, v_sb=v_sb, ot_sb=ot_sb):
                for p in pend:
                    attn_stage2(v_sb, ot_sb, *p)

            if pair + 2 < npair:
                qkv[pair + 2] = emit_qkv(pair + 2, mid=backlog)
            else:
                backlog()
                if pair + 1 < npair:
                    # the tail is exp-throughput-bound on Act: pre-emit the
                    # last pair's first stage1 blocks so their exps overlap
                    # this pair's projection instead of serializing after it
                    nqk, nv = qkv[pair + 1]
                    carry[pair + 1] = emit_attn(nqk, nv, end_block=8)
                # no qkv to overlap at the tail: pad so the last projections
                # don't idle the PE (and drop its p-state) while the final
                # d-chain completes
                emit_fill(8)
            emit_proj(pair, ot_sb, last=(pair == npair - 1))

    nc.compile()
    return nc


def _prep_weights(qkv_w, qkv_b, proj_w, proj_b, attention_biases, bias_idxs):
    perm_qk = np.empty(512, dtype=np.int64)
    for jp in range(512):
        jt, r = divmod(jp, 128)
        g, d = divmod(r, 32)
        if jt < 2:
            perm_qk[jp] = (jt * 4 + g) * 192 + d
        else:
            perm_qk[jp] = ((jt - 2) * 4 + g) * 192 + 32 + d
    perm_v = np.empty(DH, dtype=np.int64)
    for jv in range(DH):
        h, dv = divmod(jv, 32 * 4)
        perm_v[jv] = h * 192 + 64 + dv

    def hi_lo(w):  # w: [rows, 512] scaled; -> hi, lo, hi/32 as fp8 f32-arrays
        # slice order (hi, lo, hi32): the kernel DMAs slices 0-1 and derives
        # slice 2 = slice0 * 2^-5 on-chip, so hi32 must be the LAST slice
        ws = w * WSCALE
        hi = ws.astype(np.float32).astype(F8)
        hif = hi.astype(np.float32)
        hi32 = (hif / 32.0).astype(F8)
        lo = (ws - hif).astype(np.float32).astype(F8)
        return hi, lo, hi32

    wq = qkv_w[perm_qk]  # [512 j', 512 c]
    parts = hi_lo(wq)
    wqk_ = np.zeros((128, 3, 4, 4, 128), dtype=F8)
    for s, p in enumerate(parts):
        # p [j', c] -> [pp, kc, jt, m]
        wqk_[:, s] = np.ascontiguousarray(
            p.reshape(4, 128, 4, 128).transpose(3, 2, 0, 1)
        )
    qkb_ = np.ascontiguousarray(qkv_b[perm_qk].reshape(4, 128)).astype(np.float32)

    wvm = qkv_w[perm_v]  # [1024 j, 512 c]
    parts = hi_lo(wvm)
    wv_ = np.zeros((128, 3, 4, DH), dtype=F8)
    for s, p in enumerate(parts):
        wv_[:, s] = np.ascontiguousarray(p.reshape(DH, 4, 128).transpose(2, 1, 0))

    pw_ = np.ascontiguousarray(
        proj_w.reshape(4, 128, 8, 128).transpose(3, 2, 0, 1).reshape(128, 4, 2, 4, 128)
    ).astype(BF16)
    bv = qkv_b[perm_v]
    pb_eff = proj_b + proj_w @ bv
    pb_ = np.ascontiguousarray(pb_eff.reshape(4, 128)).astype(np.float32)

    bias_full = attention_biases[:, bias_idxs]  # [H, query, key]
    bkq = np.transpose(bias_full, (0, 2, 1)).astype(np.float64)  # [H, key, query]
    bseed = bkq / SCALE
    bs = bseed.reshape(NUM_HEADS, 2, CH, N_TOK).transpose(2, 0, 1, 3)  # [98, H, 2, 196]
    bp_ = np.zeros((CH, 2, NUM_HEADS, TP), dtype=np.float64)
    bp_[:, 0] = bs.reshape(CH, NUM_HEADS, TP)
    bp_ = bp_.astype(F8)
    id_ = np.zeros((CH, 2, 112), dtype=F8)
    id_[:, 0, :CH][np.arange(CH), np.arange(CH)] = 1.0

    return dict(wqk=wqk_, qkb=qkb_, wv=wv_, pw=pw_, pb=pb_, bp=bp_, idT=id_)


def _prep_x_core(x_core):
    npair = x_core.shape[0] // 2
    xt = x_core.reshape(npair, TP, 4, 128).transpose(3, 0, 2, 1)  # [128, np, 4, 392]
    out = np.zeros((128, npair, 2, 4, 400), dtype=F8)
    hi = xt.astype(np.float32).astype(F8)
    out[:, :, 0, :, :TP] = hi
    out[:, :, 1, :, :TP] = ((xt - hi.astype(np.float32)) * 32.0).astype(F8)
    return out


def _unshard_y(y_core, npair=NPAIR):
    y = np.ascontiguousarray(np.transpose(y_core, (1, 3, 2, 0)))
    return y.reshape(npair * 2, N_TOK, C)


def kernel(x, qkv_w, qkv_b, proj_w, proj_b, attention_biases, bias_idxs):
    x = np.asarray(x, dtype=np.float32)
    qkv_w = np.asarray(qkv_w, dtype=np.float32)
    qkv_b = np.asarray(qkv_b, dtype=np.float32)
    proj_w = np.asarray(proj_w, dtype=np.float32)
    proj_b = np.asarray(proj_b, dtype=np.float32)
    attention_biases = np.asarray(attention_biases, dtype=np.float32)
    bias_idxs = np.asarray(bias_idxs)

    if "nc" not in _cache:
        _cache["nc"] = _build_program()
    nc = _cache["nc"]

    wmap = _prep_weights(qkv_w, qkv_b, proj_w, proj_b, attention_biases, bias_idxs)
    in_maps = []
    for core in range(N_CORES):
        m = dict(wmap)
        m["xT"] = _prep_x_core(x[core * BPC : (core + 1) * BPC])
        in_maps.append(m)

    import os

    guard = {}
    try:
        from antenv import axon_hooks  # noqa: F401
    except ImportError:
        if os.environ.get("BASS_TRACE") and not os.environ.get("BASS_NEVER_TRACE"):
            guard["BASS_NEVER_TRACE"] = True
            os.environ["BASS_NEVER_TRACE"] = "1"
    try:
        res = run_bass_kernel_spmd(nc, in_maps, list(range(N_CORES)))
    finally:
        if guard:
            os.environ.pop("BASS_NEVER_TRACE", None)
    _cache["last_res"] = res
    out = np.concatenate(
        [_unshard_y(res.results[i]["yT"]) for i in range(N_CORES)], axis=0
    )
    return out.astype(np.float32)


if __name__ == "__main__":
    print("building program...")
    _build_program(npair=2)
    print("ok")



# revision 77
# speedup vs baseline: 1.0653x; 1.0014x over previous
"""Trainium2 Bass kernel v4 for AttentionWithBias (LeViT-style attention).

Data-parallel over batch across 8 NeuronCores (32 batches/core, 16 pairs).
Cost-model-driven structure (342.7us -> 322.1us vs the v3 baseline):

  - qk/v projections: 3-term hi/lo fp8e4 DoubleRow (x_hi@w_hi + x_hi@w_lo
    + x_lo@w_hi/32) -> 0.75x bf16 PE cost at ~fp32 accuracy. wqk's w_hi/32
    slice is derived on-chip (exact) to shorten the serial start DMA pipe;
    wv arrives in DH-halves (hi/lo then hi32 per half) in exact need-order.
  - attention bias seeded into PSUM by fp8 DoubleRow identity matmul.
  - S^T and O^T in bf16 (98/98 key chunks; fp8 fails the error budget here).
  - per-head 1-bank S PSUM tiles (3 bufs) free a bank so the shared
    qkv/proj PSUM ring can triple-buffer -> no evacuation stalls.
  - softmax denominator: fp16 chunk-add on DVE (2x path, exact here) +
    partition_all_reduce on GPSIMD; normalize: DVE reciprocal + multiply.
    (DVE ALU `divide` would fuse these but the executor rejects it.)
  - v-bias/proj-bias folded on host: pb_eff = proj_b + proj_w @ bv.
  - software-pipelined emission: stage2 lags stage1 by LAG blocks; the
    stage2 backlog is spliced between the next pair's qk and v sections;
    qkv is prefetched two pairs deep at the start.
  - all DMAs on the SP queue in exact need-order (the cost model runs one
    serial DMA pipe); PE warm-up and fill matmuls bridge DMA waits so the
    p-state never drops; the last pair ships y per-ct to shorten the drain.
  - the last pair's whole attention is pre-emitted during the previous
    pair's projection window: the tail is exp-throughput-bound on Act, so
    its 16 exps start ~5us earlier and the final projection never stalls.
"""

import sys

sys.path.insert(0, "/opt/trn_rl_repo")

from contextlib import ExitStack

import numpy as np
import ml_dtypes

import concourse.bacc as bacc
import concourse.tile as tile
import concourse.mybir as mybir
import concourse.bass_isa as bass_isa
from concourse.bass_utils import run_bass_kernel_spmd

BF16 = ml_dtypes.bfloat16
F8 = ml_dtypes.float8_e4m3fn
DR = mybir.MatmulPerfMode.DoubleRow

B, N_TOK, C = 256, 196, 512
NUM_HEADS, KEY_DIM, D_V = 8, 32, 128
DH = D_V * NUM_HEADS  # 1024
SCALE = KEY_DIM ** (-0.5)

N_CORES = 8
BPC = B // N_CORES  # 32
NPAIR = BPC // 2  # 16
TP = 2 * N_TOK  # 392
CH = 98  # key chunk

WSCALE = 64.0
N_WARM = 42
LAG = 5

_cache = {}


def _build_program(npair=NPAIR):
    nc = bacc.Bacc("TRN2", target_bir_lowering=False, debug=False)
    f32 = mybir.dt.float32
    f16 = mybir.dt.float16
    bf16 = mybir.dt.bfloat16
    fp8 = mybir.dt.float8e4

    # ---- DRAM I/O ----
    xT = nc.dram_tensor("xT", [128, npair, 2, 4, 400], fp8, kind="ExternalInput").ap()
    wqk = nc.dram_tensor("wqk", [128, 3, 4, 4, 128], fp8, kind="ExternalInput").ap()
    qkb = nc.dram_tensor("qkb", [4, 128], f32, kind="ExternalInput").ap()
    wv = nc.dram_tensor("wv", [128, 3, 4, DH], fp8, kind="ExternalInput").ap()
    pw = nc.dram_tensor("pw", [128, 4, 2, 4, 128], bf16, kind="ExternalInput").ap()
    pb = nc.dram_tensor("pb", [4, 128], f32, kind="ExternalInput").ap()
    idT = nc.dram_tensor("idT", [CH, 2, 112], fp8, kind="ExternalInput").ap()
    bp = nc.dram_tensor("bp", [CH, 2, NUM_HEADS, TP], fp8, kind="ExternalInput").ap()
    yT = nc.dram_tensor("yT", [128, npair, 4, TP], f32, kind="ExternalOutput").ap()

    with tile.TileContext(nc) as tc, ExitStack() as ctx:
        consts = ctx.enter_context(tc.tile_pool(name="consts", bufs=1))
        xio = ctx.enter_context(tc.tile_pool(name="xio", bufs=3))
        qkp = ctx.enter_context(tc.tile_pool(name="qkp", bufs=3))
        vp = ctx.enter_context(tc.tile_pool(name="vp", bufs=3))
        ep = ctx.enter_context(tc.tile_pool(name="ep", bufs=7))
        dp = ctx.enter_context(tc.tile_pool(name="dp", bufs=11))
        rp = ctx.enter_context(tc.tile_pool(name="rp", bufs=5))
        op = ctx.enter_context(tc.tile_pool(name="op", bufs=2))
        yp = ctx.enter_context(tc.tile_pool(name="yp", bufs=2))
        ps = ctx.enter_context(tc.tile_pool(name="ps", bufs=1, space="PSUM"))

        # ---- constants ----
        # The cost model runs all DMA transfers through ONE serial pipe, in
        # HWDGE arrival order. Everything goes on the SP queue in exactly the
        # order the PE will need it: wqk -> x(pair0) -> qkb -> wv(hi,lo) ->
        # wv(hi32) -> x(pair1) -> bias tables -> proj weights.
        wqk_sb = consts.tile([128, 3, 4, 4, 128], fp8)
        nc.sync.dma_start(out=wqk_sb[:, :2], in_=wqk[:, :2])
        xp0_sb = xio.tile([128, 2, 4, 400], fp8, name="xp")
        nc.sync.dma_start(out=xp0_sb, in_=xT[:, 0])
        qkb_sb = consts.tile([128, 4], f32)
        nc.sync.dma_start(out=qkb_sb, in_=qkb.rearrange("k p -> p k"))
        wv_sb = consts.tile([128, 3, 4, DH], fp8)
        # f-halves in need-order: each half's hi/lo slices then its hi32
        # slice, so the (f-major-ordered) pair-0 v matmuls start as soon as
        # the first half lands on the serial DMA pipe
        nc.sync.dma_start(out=wv_sb[:, :2, :, :512], in_=wv[:, :2, :, :512])
        nc.sync.dma_start(out=wv_sb[:, 2, :, :512], in_=wv[:, 2, :, :512])
        nc.sync.dma_start(out=wv_sb[:, :2, :, 512:], in_=wv[:, :2, :, 512:])
        nc.sync.dma_start(out=wv_sb[:, 2, :, 512:], in_=wv[:, 2, :, 512:])
        # wqk's hi32 = hi * 2^-5 is derived on-chip instead (exact: fp8 ->
        # f32 -> scale -> fp8 round matches the host computation), keeping
        # the pipe shorter before x arrives
        nc.vector.tensor_scalar_mul(
            out=wqk_sb[:, 2], in0=wqk_sb[:, 0], scalar1=1.0 / 32.0
        )
        bp_sb = consts.tile([CH, 2, NUM_HEADS, TP], fp8)
        nc.sync.dma_start(out=bp_sb, in_=bp)
        id_sb = consts.tile([CH, 2, 112], fp8)
        nc.sync.dma_start(out=id_sb, in_=idT)
        xp1_sb = xio.tile([128, 2, 4, 400], fp8, name="xp")
        nc.sync.dma_start(out=xp1_sb, in_=xT[:, 1])
        pw_sb = consts.tile([128, 4, 2, 4, 128], bf16)
        nc.sync.dma_start(out=pw_sb, in_=pw)
        pb_sb = consts.tile([128, 4], f32)
        nc.sync.dma_start(out=pb_sb, in_=pb.rearrange("k p -> p k"))
        wm_sb = consts.tile([128, 128], bf16)
        nc.vector.memset(wm_sb, 1.0)

        # PE warm-up: p-state ramps to full rate after ~3us of continuous busy.
        def emit_fill(n):
            warm_ps = ps.tile([128, 512], f32, tag="ps1", bufs=3, name="warm_ps")
            for w in range(n):
                nc.tensor.matmul(
                    warm_ps[:, :128], lhsT=wm_sb, rhs=wm_sb, start=True, stop=True
                )

        emit_fill(N_WARM)

        def emit_qkv(pair, mid=None, xp=None, fill_after_qk=0):
            if xp is None:
                xp = xio.tile([128, 2, 4, 400], fp8, name="xp")
                nc.sync.dma_start(out=xp, in_=xT[:, pair])

            # ---- qk projection: 3-term fp8 DR ----
            qk_sb = qkp.tile([128, 5, 400], bf16, name="qk_sb")
            # (x hi/lo, w hi/lo/hi32); the hi32 term last so the derived wv
            # hi32 slice isn't needed until each gen's end
            terms = [(0, 0), (0, 1), (1, 2)]
            for jt in range(4):
                qk_ps = ps.tile([128, TP], f32, tag="ps1", bufs=3, name="qk_ps", padded_shape=(..., 512))
                n = 0
                for xs, wsd in terms:
                    for t in range(2):
                        nc.tensor.matmul(
                            qk_ps,
                            lhsT=wqk_sb[:, wsd, 2 * t : 2 * t + 2, jt, :],
                            rhs=xp[:, xs, 2 * t : 2 * t + 2, :TP],
                            start=(n == 0),
                            stop=(n == 5),
                            perf_mode=DR,
                        )
                        n += 1
                nc.vector.tensor_scalar(
                    out=qk_sb[:, jt, :TP],
                    in0=qk_ps,
                    scalar1=1.0 / WSCALE,
                    scalar2=qkb_sb[:, jt : jt + 1],
                    op0=mybir.AluOpType.mult,
                    op1=mybir.AluOpType.add,
                )

            if mid is not None:
                # the previous pair's stage2 backlog slots in here, so its
                # DVE norms land ahead of the v evacuations in DVE's queue
                mid()
            if fill_after_qk:
                # pair 0 only: the wv DMA is still in flight when the qk
                # matmuls finish; keep the PE p-state warm until it lands
                emit_fill(fill_after_qk)

            # ---- v projection: 3-term fp8 DR ----
            # pair 0 runs f-major (all DH-first-half gens first) to match the
            # wv DMA/derive split landing order; later pairs keep cc-major,
            # which schedules better against the stage2 backlog
            v_sb = vp.tile([CH, 4, DH], bf16, name="v_sb")
            gens = (
                [(f, cc) for f in range(2) for cc in range(4)]
                if pair == 0
                else [(f, cc) for cc in range(4) for f in range(2)]
            )
            for f, cc in gens:
                    tok0 = CH * cc
                    v_ps = ps.tile([CH, 512], f32, tag="ps1", bufs=3, name="v_ps")
                    n = 0
                    for xs, wsd in terms:
                        for t in range(2):
                            nc.tensor.matmul(
                                v_ps,
                                lhsT=xp[:, xs, 2 * t : 2 * t + 2, tok0 : tok0 + CH],
                                rhs=wv_sb[:, wsd, 2 * t : 2 * t + 2, 512 * f : 512 * (f + 1)],
                                start=(n == 0),
                                stop=(n == 5),
                                perf_mode=DR,
                            )
                            n += 1
                    vdst = v_sb[:, cc, 512 * f : 512 * (f + 1)]
                    nc.vector.tensor_scalar_mul(out=vdst, in0=v_ps, scalar1=1.0 / WSCALE)
            return qk_sb, v_sb

        eb_count = [0]

        def attn_stage1(qk_sb, i, hp):
            t0 = N_TOK * i
            e_sb = ep.tile([128, 2, 2, 208], bf16, name="e_sb")
            if eb_count[0] < 7:
                # zero partitions 96..127 on every ring slot (ep bufs=7) so the
                # 128-channel partition reduction reads zeros there; later
                # generations reuse the same bytes untouched. Pool is idle in
                # the ramp-up phase, so these don't block the evac engines.
                nc.gpsimd.memset(e_sb[96:, :, :, :], 0.0)
                eb_count[0] += 1
            for hh in range(2):
                h = 2 * hp + hh
                # per-head 1-bank PSUM tile (vs 2-bank for the head pair):
                # frees a PSUM bank so the shared ps1 ring can triple-buffer
                s_ps = ps.tile([CH, 2, N_TOK], f32, tag="psS", bufs=3, name="s_ps", padded_shape=(..., 208))
                nc.tensor.matmul(
                    s_ps,
                    lhsT=id_sb[:, :, :CH],
                    rhs=bp_sb[:, :, h, :],
                    start=True,
                    stop=False,
                    perf_mode=DR,
                )
                g = h % 4
                jq = h // 4
                jk = 2 + h // 4
                p0 = 32 * g
                q_rhs = qk_sb[p0 : p0 + 32, jq, t0 : t0 + N_TOK]
                for cc in range(2):
                    k_lhs = qk_sb[p0 : p0 + 32, jk, t0 + CH * cc : t0 + CH * (cc + 1)]
                    nc.tensor.matmul(
                        s_ps[:, cc, :],
                        lhsT=k_lhs,
                        rhs=q_rhs,
                        start=False,
                        stop=True,
                        skip_group_check=True,
                        tile_position=(p0, 0),
                    )
                nc.scalar.activation(
                    out=e_sb[:CH, hh, :, :N_TOK],
                    in_=s_ps,
                    func=mybir.ActivationFunctionType.Exp,
                    scale=SCALE,
                )
            # denominator: chunk-add on DVE (all-2-byte operands hit the 2x
            # path; fp16 keeps ~11 mantissa bits and cannot overflow here),
            # then partition all-reduce on Pool (f32 accumulate)
            ee_sb = dp.tile([128, 2, N_TOK], f16, name="ee_sb")
            nc.vector.tensor_add(
                out=ee_sb, in0=e_sb[:, :, 0, :N_TOK], in1=e_sb[:, :, 1, :N_TOK]
            )
            d_sb = dp.tile([128, 2, N_TOK], f32, name="d_sb")
            nc.gpsimd.partition_all_reduce(
                out_ap=d_sb, in_ap=ee_sb, channels=128, reduce_op=bass_isa.ReduceOp.add
            )
            return e_sb, d_sb

        def attn_stage2(v_sb, ot_sb, e_sb, d_sb, i, hp):
            t0 = N_TOK * i
            o_ps = ps.tile([128, 2, N_TOK], f32, tag="psOD", bufs=2, name="o_ps", padded_shape=(..., 256))
            for hh in range(2):
                h = 2 * hp + hh
                for cc in range(2):
                    nc.tensor.matmul(
                        o_ps[:, hh, :],
                        lhsT=v_sb[:, 2 * i + cc, 128 * h : 128 * (h + 1)],
                        rhs=e_sb[:CH, hh, cc, :N_TOK],
                        start=(cc == 0),
                        stop=(cc == 1),
                    )
            rec_sb = rp.tile([128, 2, N_TOK], f32, name="rec_sb")
            nc.vector.reciprocal_approx_fast(out=rec_sb, in_=d_sb)
            nc.vector.tensor_mul(
                out=ot_sb[:, 2 * hp : 2 * hp + 2, t0 : t0 + N_TOK],
                in0=o_ps,
                in1=rec_sb,
            )

        def emit_attn(qk_sb, v_sb, ot_sb=None, pend=None, start_block=0, end_block=8):
            if ot_sb is None:
                ot_sb = op.tile([128, NUM_HEADS, 400], bf16, name="ot_sb")
            if pend is None:
                pend = []
            blocks = [(i, hp) for i in range(2) for hp in range(NUM_HEADS // 2)]
            for i, hp in blocks[start_block:end_block]:
                e_sb, d_sb = attn_stage1(qk_sb, i, hp)
                pend.append((e_sb, d_sb, i, hp))
                if len(pend) > LAG - 1:
                    attn_stage2(v_sb, ot_sb, *pend.pop(0))
            return ot_sb, pend

        def emit_proj(pair, ot_sb, last=False):
            y_sb = yp.tile([128, 4, TP], f32, name="y_sb")
            for ct in range(4):
                p_ps = ps.tile([128, TP], f32, tag="ps1", bufs=3, name="p_ps", padded_shape=(..., 512))
                for jc in range(8):
                    nc.tensor.matmul(
                        p_ps,
                        lhsT=pw_sb[:, jc // 2, jc % 2, ct, :],
                        rhs=ot_sb[:, jc, :TP],
                        start=(jc == 0),
                        stop=(jc == 7),
                    )
                nc.scalar.activation(
                    out=y_sb[:, ct, :],
                    in_=p_ps,
                    func=mybir.ActivationFunctionType.Identity,
                    bias=pb_sb[:, ct : ct + 1],
                )
                if last:
                    # drain the tail: ship each ct slice as soon as it lands
                    nc.sync.dma_start(
                        out=yT[:, pair, ct : ct + 1], in_=y_sb[:, ct : ct + 1]
                    )
            if not last:
                nc.sync.dma_start(out=yT[:, pair], in_=y_sb)

        qkv = {
            0: emit_qkv(0, xp=xp0_sb, fill_after_qk=0),
            1: emit_qkv(1, xp=xp1_sb),
        }
        carry = {}
        for pair in range(npair):
            qk_sb, v_sb = qkv.pop(pair)
            if pair in carry:
                c_ot, c_pend = carry.pop(pair)
                ot_sb, pend = emit_attn(
                    qk_sb, v_sb, ot_sb=c_ot, pend=c_pend, start_block=8
                )
            else:
                ot_sb, pend = emit_attn(qk_sb, v_sb)

            def backlog(pend=pend, v_sb=v_sb, ot_sb=ot_sb):
                for p in pend:
                    attn_stage2(v_sb, ot_sb, *p)

            def backlog_front(pend=pend, v_sb=v_sb, ot_sb=ot_sb):
                # only half the backlog between qk and v: the other half's
                # DVE norms would delay the last v evacuations past the
                # point where the projection's PSUM ring needs them
                for p in pend[:2]:
                    attn_stage2(v_sb, ot_sb, *p)

            if pair + 2 < npair:
                qkv[pair + 2] = emit_qkv(pair + 2, mid=backlog_front)
                for p in pend[2:]:
                    attn_stage2(v_sb, ot_sb, *p)
            else:
                backlog()
                if pair + 1 < npair:
                    # the tail is exp-throughput-bound on Act: pre-emit the
                    # last pair's first stage1 blocks so their exps overlap
                    # this pair's projection instead of serializing after it
                    nqk, nv = qkv[pair + 1]
                    carry[pair + 1] = emit_attn(nqk, nv, end_block=8)
                # no qkv to overlap at the tail: pad so the last projections
                # don't idle the PE (and drop its p-state) while the final
                # d-chain completes
                emit_fill(8)
            emit_proj(pair, ot_sb, last=(pair == npair - 1))

    nc.compile()
    return nc


def _prep_weights(qkv_w, qkv_b, proj_w, proj_b, attention_biases, bias_idxs):
    perm_qk = np.empty(512, dtype=np.int64)
    for jp in range(512):
        jt, r = divmod(jp, 128)
        g, d = divmod(r, 32)
        if jt < 2:
            perm_qk[jp] = (jt * 4 + g) * 192 + d
        else:
            perm_qk[jp] = ((jt - 2) * 4 + g) * 192 + 32 + d
    perm_v = np.empty(DH, dtype=np.int64)
    for jv in range(DH):
        h, dv = divmod(jv, 32 * 4)
        perm_v[jv] = h * 192 + 64 + dv

    def hi_lo(w):  # w: [rows, 512] scaled; -> hi, lo, hi/32 as fp8 f32-arrays
        # slice order (hi, lo, hi32): the kernel DMAs slices 0-1 and derives
        # slice 2 = slice0 * 2^-5 on-chip, so hi32 must be the LAST slice
        ws = w * WSCALE
        hi = ws.astype(np.float32).astype(F8)
        hif = hi.astype(np.float32)
        hi32 = (hif / 32.0).astype(F8)
        lo = (ws - hif).astype(np.float32).astype(F8)
        return hi, lo, hi32

    wq = qkv_w[perm_qk]  # [512 j', 512 c]
    parts = hi_lo(wq)
    wqk_ = np.zeros((128, 3, 4, 4, 128), dtype=F8)
    for s, p in enumerate(parts):
        # p [j', c] -> [pp, kc, jt, m]
        wqk_[:, s] = np.ascontiguousarray(
            p.reshape(4, 128, 4, 128).transpose(3, 2, 0, 1)
        )
    qkb_ = np.ascontiguousarray(qkv_b[perm_qk].reshape(4, 128)).astype(np.float32)

    wvm = qkv_w[perm_v]  # [1024 j, 512 c]
    parts = hi_lo(wvm)
    wv_ = np.zeros((128, 3, 4, DH), dtype=F8)
    for s, p in enumerate(parts):
        wv_[:, s] = np.ascontiguousarray(p.reshape(DH, 4, 128).transpose(2, 1, 0))

    pw_ = np.ascontiguousarray(
        proj_w.reshape(4, 128, 8, 128).transpose(3, 2, 0, 1).reshape(128, 4, 2, 4, 128)
    ).astype(BF16)
    bv = qkv_b[perm_v]
    pb_eff = proj_b + proj_w @ bv
    pb_ = np.ascontiguousarray(pb_eff.reshape(4, 128)).astype(np.float32)

    bias_full = attention_biases[:, bias_idxs]  # [H, query, key]
    bkq = np.transpose(bias_full, (0, 2, 1)).astype(np.float64)  # [H, key, query]
    bseed = bkq / SCALE
    bs = bseed.reshape(NUM_HEADS, 2, CH, N_TOK).transpose(2, 0, 1, 3)  # [98, H, 2, 196]
    bp_ = np.zeros((CH, 2, NUM_HEADS, TP), dtype=np.float64)
    bp_[:, 0] = bs.reshape(CH, NUM_HEADS, TP)
    bp_ = bp_.astype(F8)
    id_ = np.zeros((CH, 2, 112), dtype=F8)
    id_[:, 0, :CH][np.arange(CH), np.arange(CH)] = 1.0

    return dict(wqk=wqk_, qkb=qkb_, wv=wv_, pw=pw_, pb=pb_, bp=bp_, idT=id_)


def _prep_x_core(x_core):
    npair = x_core.shape[0] // 2
    xt = x_core.reshape(npair, TP, 4, 128).transpose(3, 0, 2, 1)  # [128, np, 4, 392]
    out = np.zeros((128, npair, 2, 4, 400), dtype=F8)
    hi = xt.astype(np.float32).astype(F8)
    out[:, :, 0, :, :TP] = hi
    out[:, :, 1, :, :TP] = ((xt - hi.astype(np.float32)) * 32.0).astype(F8)
    return out


def _unshard_y(y_core, npair=NPAIR):
    y = np.ascontiguousarray(np.transpose(y_core, (1, 3, 2, 0)))
    return y.reshape(npair * 2, N_TOK, C)


def kernel(x, qkv_w, qkv_b, proj_w, proj_b, attention_biases, bias_idxs):
    x = np.asarray(x, dtype=np.float32)
    qkv_w = np.asarray(qkv_w, dtype=np.float32)
    qkv_b = np.asarray(qkv_b, dtype=np.float32)
    proj_w = np.asarray(proj_w, dtype=np.float32)
    proj_b = np.asarray(proj_b, dtype=np.float32)
    attention_biases = np.asarray(attention_biases, dtype=np.float32)
    bias_idxs = np.asarray(bias_idxs)

    if "nc" not in _cache:
        _cache["nc"] = _build_program()
    nc = _cache["nc"]

    wmap = _prep_weights(qkv_w, qkv_b, proj_w, proj_b, attention_biases, bias_idxs)
    in_maps = []
    for core in range(N_CORES):
        m = dict(wmap)
        m["xT"] = _prep_x_core(x[core * BPC : (core + 1) * BPC])
        in_maps.append(m)

    import os

    guard = {}
    try:
        from antenv import axon_hooks  # noqa: F401
    except ImportError:
        if os.environ.get("BASS_TRACE") and not os.environ.get("BASS_NEVER_TRACE"):
            guard["BASS_NEVER_TRACE"] = True
            os.environ["BASS_NEVER_TRACE"] = "1"
    try:
        res = run_bass_kernel_spmd(nc, in_maps, list(range(N_CORES)))
    finally:
        if guard:
            os.environ.pop("BASS_NEVER_TRACE", None)
    _cache["last_res"] = res
    out = np.concatenate(
        [_unshard_y(res.results[i]["yT"]) for i in range(N_CORES)], axis=0
    )
    return out.astype(np.float32)


if __name__ == "__main__":
    print("building program...")
    _build_program(npair=2)
    print("ok")

